# revision 20
# baseline (speedup 1.0000x reference)
"""Multi-head attention Trainium2 kernel, 8-core SPMD (v2, fp8 DoubleRow).

Problem: x[2,4096,512], 8 heads of 64; per-head QKV proj, softmax(QK^T/8)V,
concat, output proj.

Sharding: sequence-parallel, no collectives. Core c handles batch b=c//4 and
query rows [1024*(c%4), ...+1024). Each core computes K/V for the full 4096-row
sequence of its batch; x is host-rolled so local query rows sit at 0:1024.

Speed structure:
- Q/K/V projections in f32r; K/Q converted to fp8-e4m3 with a host-side
  weight-column permutation so the PSUM->SBUF convert is partition-preserving
  and lands directly in the DoubleRow plane layout.
- Scores and PV use fp8-e4m3 DoubleRow matmuls (0.5 cyc/row): score stationary
  kT8[32,2,128] / moving qT8[32,2,512]; PV stationary vA[128,2,66] (a ones
  column provides the softmax denominator; a zero pad column satisfies the
  dual-fp8 Ldweights even-column ISA restriction) / moving pt[128,2,512].
- softmax exp is the wall; it is split across two engines by t-tile pair:
  lane A: ACT native Exp -> fp8 (scale=0.125 fused)
  lane D: DVE tensor_scalar computes e4m3 BITS of exp directly
          (Schraudolph: round(A*s+B) -> int8 -> bitcast fp8e4)
  (Pool/DMA cannot read PSUM, so no third lane is possible.)
- Attention is streamed in two t-groups (pairs 0-6, 7-15); K/V/Q projection of
  chunks 4..7 interleaves into group-A block boundaries at psc-ring-tile
  granularity; per-(h,sc) PV accumulates in one PSUM bank per group, group A
  drained to SBUF (yacc) and merged + normalized at group-B block end. PSUM:
  3x2-bank score/proj ring + 2 pav accumulators = 8 banks.
"""

import numpy as np

import concourse.bass as bass
from concourse import bacc
import concourse.mybir as mybir
import concourse.tile as tile
from concourse.bass_utils import run_bass_kernel_spmd

F32 = mybir.dt.float32
F32R = mybir.dt.float32r
BF16 = mybir.dt.bfloat16
FP8 = mybir.dt.float8e4
I8 = mybir.dt.int8
DR = mybir.MatmulPerfMode.DoubleRow
ADD = mybir.AluOpType.add
MULT = mybir.AluOpType.mult

B, S, D, H, E = 2, 4096, 512, 8, 64
NCORES = 8
QCHUNK = S // 4          # 1024 query rows per core
TCH = 512                # t-rows per projection chunk
NPAIR = S // 256         # 16 t-tile pairs
GROUP_A = list(range(7))        # chunks 0..3 (pair 6 needs chunk 3)
GROUP_B = list(range(7, NPAIR)) # chunks 3..7

A_SCH = 1.442695041      # 8*log2(e)*0.125
B_SCH = 55.632           # calibrated: max rel err 7.2% per weight

# exp lane pattern, cycled over all 256 (h,sc,pair) tasks: A=ACT native exp,
# D=DVE Schraudolph. (Pool cannot read PSUM and DMA cannot read PSUM, so a
# Pool lane would cost DVE the same feeder op as doing the exp on DVE.)
LANES = "DADADAADADAADADAADADAADADAADADAA"  # 19 A / 13 D per 32


def build_program():
    nc = bacc.Bacc()
    xt_d = nc.dram_tensor("xt", [D, S], F32R, kind="ExternalInput")
    wq_d = nc.dram_tensor("wq", [128, 4, 512], F32R, kind="ExternalInput")
    wk_d = nc.dram_tensor("wk", [128, 4, 512], F32R, kind="ExternalInput")
    wv_d = nc.dram_tensor("wv", [128, 4, 512], F32R, kind="ExternalInput")
    wo_d = nc.dram_tensor("wo", [64, 8, 512], F32R, kind="ExternalInput")
    bq_d = nc.dram_tensor("bq", [128, 4], F32, kind="ExternalInput")
    bk_d = nc.dram_tensor("bk", [128, 4], F32, kind="ExternalInput")
    bv_d = nc.dram_tensor("bv", [512], F32, kind="ExternalInput")
    bo_d = nc.dram_tensor("bo", [512], F32, kind="ExternalInput")
    out_d = nc.dram_tensor("out", [QCHUNK, D], F32, kind="ExternalOutput")

    with tile.TileContext(nc) as tc:
        with (
            tc.tile_pool(name="const", bufs=1) as cpool,
            tc.tile_pool(name="work", bufs=4) as wpool,
            tc.tile_pool(name="xtp", bufs=2) as xpool,
            tc.tile_pool(name="psc", bufs=3, space="PSUM") as pscpool,
            tc.tile_pool(name="pav", bufs=2, space="PSUM") as pavpool,
            tc.tile_pool(name="dr", bufs=2, space="DRAM") as dpool,
        ):
            wq_s = cpool.tile([128, 4, 512], F32R, tag="wq")
            wk_s = cpool.tile([128, 4, 512], F32R, tag="wk")
            wv_s = cpool.tile([128, 4, 512], F32R, tag="wv")
            wo_s = cpool.tile([64, 8, 512], F32R, tag="wo")
            bq_s = cpool.tile([128, 4], F32, tag="bq")
            bk_s = cpool.tile([128, 4], F32, tag="bk")
            bv_r = cpool.tile([128, 512], F32, tag="bvr")
            bo_r = cpool.tile([128, 512], F32, tag="bor")
            nc.sync.dma_start(wk_s[:], wk_d[:])
            nc.sync.dma_start(bk_s[:], bk_d[:])
            nc.sync.dma_start(wv_s[:], wv_d[:])
            nc.sync.dma_start(bv_r[:], bv_d[:].unsqueeze(0).to_broadcast((128, 512)))
            nc.sync.dma_start(wq_s[:], wq_d[:])
            nc.sync.dma_start(bq_s[:], bq_d[:])
            nc.sync.dma_start(bo_r[:], bo_d[:].unsqueeze(0).to_broadcast((128, 512)))
            nc.sync.dma_start(wo_s[:], wo_d[:])

            # fp8 operand tensors
            kT8 = cpool.tile([128, 4, S], FP8, tag="kT8")
            qT8 = cpool.tile([128, 4, QCHUNK], FP8, tag="qT8")
            vA = cpool.tile([128, S // 128, H, E + 2], FP8, tag="vA")
            yacc = cpool.tile([65, 16, 512], F32, tag="yacc")
            yT2 = cpool.tile([64, H, QCHUNK], F32R, tag="yT2")
            nc.vector.memset(vA[:, :, :, E], 1.0)
            nc.vector.memset(vA[:, :, :, E + 1], 0.0)

            def make_proj_tasks(ch):
                """Per-chunk projection split into psc-ring-sized tasks so it
                can interleave with attention blocks at fine grain."""
                state = {}

                def t_x():
                    xT = xpool.tile([128, 4, TCH], F32R, tag="xT")
                    for ds in range(4):
                        nc.sync.dma_start(
                            xT[:, ds, :],
                            xt_d[ds * 128:(ds + 1) * 128,
                                 ch * TCH:(ch + 1) * TCH])
                    state["xT"] = xT

                def t_k(ega):
                    def f():
                        xT = state["xT"]
                        pk = pscpool.tile([128, 2, 512], F32, tag="psc")
                        for i in range(2):
                            eg = ega * 2 + i
                            for ds in range(4):
                                nc.tensor.matmul(
                                    pk[:, i, :],
                                    wk_s[:, ds, eg * 128:(eg + 1) * 128],
                                    xT[:, ds, :], start=(ds == 0),
                                    stop=(ds == 3))
                            nc.scalar.activation(
                                kT8[:, eg, ch * TCH:(ch + 1) * TCH],
                                pk[:, i, :],
                                mybir.ActivationFunctionType.Identity,
                                bias=bk_s[:, eg:eg + 1], scale=1.0)
                    return f

                def t_v(tsa):
                    def f():
                        xT = state["xT"]
                        pv = pscpool.tile([128, 2, 512], F32, tag="psc")
                        for i in range(2):
                            ts = tsa * 2 + i
                            for ds in range(4):
                                nc.tensor.matmul(
                                    pv[:, i, :],
                                    xT[:, ds, ts * 128:(ts + 1) * 128],
                                    wv_s[:, ds, :], start=(ds == 0),
                                    stop=(ds == 3))
                            nc.vector.tensor_tensor(
                                out=vA[:, ch * 4 + ts, :, 0:E],
                                in0=pv[:, i, :].rearrange("p (h e) -> p h e",
                                                          h=H),
                                in1=bv_r[:].rearrange("p (h e) -> p h e", h=H),
                                op=ADD)
                    return f

                def t_q(ega):
                    def f():
                        xT = state["xT"]
                        pq = pscpool.tile([128, 2, 512], F32, tag="psc")
                        for i in range(2):
                            eg = ega * 2 + i
                            for ds in range(4):
                                nc.tensor.matmul(
                                    pq[:, i, :],
                                    wq_s[:, ds, eg * 128:(eg + 1) * 128],
                                    xT[:, ds, :], start=(ds == 0),
                                    stop=(ds == 3))
                            nc.vector.tensor_scalar(
                                qT8[:, eg, ch * TCH:(ch + 1) * TCH],
                                pq[:, i, :], bq_s[:, eg:eg + 1], None, ADD)
                    return f

                def first():
                    t_x()
                    t_k(0)()

                if ch < QCHUNK // TCH:
                    # queries first so attention can start early
                    tasks = [first, t_q(0), t_v(0), t_k(1), t_v(1), t_q(1)]
                else:
                    tasks = [first, t_v(0), t_k(1), t_v(1)]
                return tasks

            def emit_proj_chunk(ch):
                for t in make_proj_tasks(ch):
                    t()

            lane_ctr = [0]

            def emit_block(h, sc, pairs, is_group_a):
                a0 = 32 * (h % 4)
                g0 = 2 * (h // 4)
                n = len(pairs)
                pav = pavpool.tile([128, 512], F32, tag="pav")

                def emit_pv(item):
                    j, tp, ptv = item
                    nc.tensor.matmul(
                        pav[0:66, :], vA[:, 2 * tp:2 * tp + 2, h, :], ptv,
                        start=(j == 0), stop=(j == n - 1), perf_mode=DR)

                pend = []
                for j, tp in enumerate(pairs):
                    lane = LANES[lane_ctr[0] % len(LANES)]
                    lane_ctr[0] += 1
                    psc = pscpool.tile([128, 2, 512], F32, tag="psc")
                    for kt in (0, 1):
                        tt = 2 * tp + kt
                        nc.tensor.matmul(
                            psc[:, kt, :],
                            kT8[a0:a0 + 32, g0:g0 + 2, tt * 128:(tt + 1) * 128],
                            qT8[a0:a0 + 32, g0:g0 + 2, sc * 512:(sc + 1) * 512],
                            start=True, stop=True, perf_mode=DR,
                            tile_position=(a0, 0))
                    if lane == "A":
                        pt = wpool.tile([128, 2, 512], FP8, tag="ptA")
                        nc.scalar.activation(
                            pt[:], psc[:], mybir.ActivationFunctionType.Exp,
                            scale=0.125)
                        ptv = pt[:]
                    elif lane == "D":
                        pti = wpool.tile([128, 2, 512], I8, tag="ptD")
                        nc.vector.tensor_scalar(
                            pti[:], psc[:], A_SCH, B_SCH, MULT, ADD)
                        ptv = pti[:].bitcast(FP8)
                    else:  # lane P: DVE copies PSUM->SBUF bf16, Pool does sch
                        scf = wpool.tile([128, 2, 512], BF16, tag="scf")
                        nc.vector.tensor_copy(scf[:], psc[:])
                        pti = wpool.tile([128, 2, 512], I8, tag="ptP")
                        nc.gpsimd.tensor_scalar(
                            pti[:], scf[:], A_SCH, B_SCH, MULT, ADD)
                        ptv = pti[:].bitcast(FP8)
                    pend.append((j, tp, ptv))
                    if len(pend) == 3:
                        emit_pv(pend.pop(0))
                for item in pend:
                    emit_pv(item)

                slot = sc * 8 + h
                if is_group_a:
                    nc.vector.tensor_copy(yacc[:, slot, :], pav[0:65, :])
                else:
                    tmp = wpool.tile([65, 512], F32, tag="tmp")
                    nc.vector.tensor_tensor(
                        out=tmp[:], in0=yacc[:, slot, :], in1=pav[0:65, :],
                        op=ADD)
                    rec = wpool.tile([1, 512], F32, tag="rec")
                    nc.vector.reciprocal(rec[:], tmp[64:65, :])
                    rec_d = dpool.tile([1, 512], F32, tag="recd")
                    nc.sync.dma_start(rec_d[:], rec[:])
                    rrep = wpool.tile([64, 512], F32, tag="rrep")
                    nc.sync.dma_start(rrep[:], rec_d[:].to_broadcast((64, 512)))
                    nc.gpsimd.tensor_tensor(
                        out=yT2[0:64, h, sc * 512:(sc + 1) * 512],
                        in0=tmp[0:64, :], in1=rrep[:], op=MULT)

            def phase3_task(sc, sta):
                def f():
                    po = pscpool.tile([128, 2, 512], F32, tag="psc")
                    for i in range(2):
                        st = sc * 4 + sta * 2 + i
                        for h in range(H):
                            nc.tensor.matmul(
                                po[:, i, :], yT2[0:64, h, st * 128:(st + 1) * 128],
                                wo_s[0:64, h, :], start=(h == 0), stop=(h == 7))
                        o_s = wpool.tile([128, 512], F32, tag="osb")
                        nc.vector.tensor_tensor(out=o_s[:, :], in0=po[:, i, :],
                                                in1=bo_r[:], op=ADD)
                        nc.sync.dma_start(out_d[st * 128:(st + 1) * 128, :],
                                          o_s[:])
                return f

            # ---- emission ----
            emit_proj_chunk(0)
            emit_proj_chunk(1)
            emit_proj_chunk(2)
            emit_proj_chunk(3)  # group A reaches pair 6 = chunk 3
            # chunks 4..7 queued as fine-grained tasks, 2 per block boundary
            proj_queue = []
            for ch in range(4, 8):
                proj_queue.extend(make_proj_tasks(ch))
            blocks = [(sc, h) for sc in (0, 1) for h in range(H)]
            for bi, (sc, h) in enumerate(blocks):
                emit_block(h, sc, GROUP_A, True)
                for _ in range(2):
                    if proj_queue:
                        proj_queue.pop(0)()
            while proj_queue:
                proj_queue.pop(0)()
            # group B; spread each sc's output projection into the next sc's
            # blocks (final sc's at the end)
            pending_p3 = []
            for sc in (0, 1):
                for h in range(H):
                    emit_block(h, sc, GROUP_B, False)
                    if pending_p3:
                        pending_p3.pop(0)()
                pending_p3 = [phase3_task(sc, 0), phase3_task(sc, 1)]
            for t in pending_p3:
                t()
    nc.compile()
    return nc


_NC = None


def _pack_weights(Wq, bq, Wk, bk, Wv, bv, Wo, bo):
    s = lambda a: np.ascontiguousarray(np.asarray(a, np.float32))
    # e-permutation for DoubleRow plane layout: column c = eg*128+p of the
    # stationary maps to head h = p//32 + 4*(eg//2), e = 32*(eg%2) + p%32
    p = np.arange(128)
    eg = np.arange(4)
    hh = p[None, :] // 32 + 4 * (eg[:, None] // 2)     # [4,128]
    ee = 32 * (eg[:, None] % 2) + p[None, :] % 32      # [4,128]

    def pack_qk(W):
        t = np.asarray(W, np.float32)[hh, :, ee]       # [4,128,512(d)]
        t = t.transpose(2, 0, 1)                       # [d, eg, p]
        t = t.reshape(4, 128, 4, 128)                  # [ds, pd, eg, p]
        return s(t.transpose(1, 0, 2, 3).reshape(128, 4, 512))

    def pack_b(b):
        return s(np.asarray(b, np.float32)[hh, ee].T)  # [128,4]

    wq_p = pack_qk(Wq)
    wk_p = pack_qk(Wk)
    bq_p = pack_b(bq)
    bk_p = pack_b(bk)
    wv_p = s(np.transpose(Wv, (1, 0, 2)).reshape(D, 512).reshape(4, 128, 512)
             .transpose(1, 0, 2))
    wo_p = s(np.asarray(Wo, np.float32).reshape(8, 64, 512).transpose(1, 0, 2))
    bv_p = s(np.asarray(bv, np.float32).reshape(512))
    bo_p = s(np.asarray(bo, np.float32))
    return dict(wq=wq_p, wk=wk_p, wv=wv_p, wo=wo_p, bq=bq_p, bk=bk_p,
                bv=bv_p, bo=bo_p)


def kernel(x, Wq, bq, Wk, bk, Wv, bv, Wo, bo, **kw):
    global _NC
    x = np.asarray(x, np.float32)
    packed = _pack_weights(Wq, bq, Wk, bk, Wv, bv, Wo, bo)

    if _NC is None:
        _NC = build_program()

    in_maps = []
    for c in range(NCORES):
        b = c // 4
        q0 = (c % 4) * QCHUNK
        xb = np.roll(x[b], -q0, axis=0)  # queries at rows 0:1024
        m = {"xt": np.ascontiguousarray(xb.T)}
        m.update(packed)
        in_maps.append(m)
    res = run_bass_kernel_spmd(_NC, in_maps, core_ids=list(range(NCORES)))
    out = np.empty((B, S, D), np.float32)
    for c in range(NCORES):
        b = c // 4
        q0 = (c % 4) * QCHUNK
        out[b, q0:q0 + QCHUNK] = res.results[c]["out"]
    return out


# revision 23
# speedup vs baseline: 1.0368x; 1.0368x over previous
"""Multi-head attention Trainium2 kernel, 8-core SPMD (v2, fp8 DoubleRow).

Problem: x[2,4096,512], 8 heads of 64; per-head QKV proj, softmax(QK^T/8)V,
concat, output proj.

Sharding: sequence-parallel, no collectives. Core c handles batch b=c//4 and
query rows [1024*(c%4), ...+1024). Each core computes K/V for the full 4096-row
sequence of its batch; x is host-rolled so local query rows sit at 0:1024.

Speed structure:
- Q/K/V projections in f32r; K/Q converted to fp8-e4m3 with a host-side
  weight-column permutation so the PSUM->SBUF convert is partition-preserving
  and lands directly in the DoubleRow plane layout.
- Scores and PV use fp8-e4m3 DoubleRow matmuls (0.5 cyc/row): score stationary
  kT8[32,2,128] / moving qT8[32,2,512]; PV stationary vA[128,2,66] (a ones
  column provides the softmax denominator; a zero pad column satisfies the
  dual-fp8 Ldweights even-column ISA restriction) / moving pt[128,2,512].
- softmax exp is the wall; it is split across two engines by t-tile pair:
  lane A: ACT native Exp -> fp8 (scale=0.125 fused)
  lane D: DVE tensor_scalar computes e4m3 BITS of exp directly
          (Schraudolph: round(A*s+B) -> int8 -> bitcast fp8e4)
  (Pool/DMA cannot read PSUM, so no third lane is possible.)
- Attention is streamed in two t-groups (pairs 0-6, 7-15); K/V/Q projection of
  chunks 4..7 interleaves into group-A block boundaries at psc-ring-tile
  granularity; per-(h,sc) PV accumulates in one PSUM bank per group, group A
  drained to SBUF (yacc) and merged + normalized at group-B block end. PSUM:
  3x2-bank score/proj ring + 2 pav accumulators = 8 banks.
"""

import numpy as np

import concourse.bass as bass
from concourse import bacc
import concourse.mybir as mybir
import concourse.tile as tile
from concourse.bass_utils import run_bass_kernel_spmd

F32 = mybir.dt.float32
F32R = mybir.dt.float32r
BF16 = mybir.dt.bfloat16
FP8 = mybir.dt.float8e4
I8 = mybir.dt.int8
DR = mybir.MatmulPerfMode.DoubleRow
ADD = mybir.AluOpType.add
MULT = mybir.AluOpType.mult

B, S, D, H, E = 2, 4096, 512, 8, 64
NCORES = 8
QCHUNK = S // 4          # 1024 query rows per core
TCH = 512                # t-rows per projection chunk
NPAIR = S // 256         # 16 t-tile pairs
GROUP_A = list(range(8))        # chunks 0..3
GROUP_B = list(range(8, NPAIR)) # chunks 4..7

A_SCH = 1.442695041      # 8*log2(e)*0.125
B_SCH = 55.632           # calibrated: max rel err 7.2% per weight

# exp lane pattern, cycled over all 256 (h,sc,pair) tasks: A=ACT native exp,
# D=DVE Schraudolph. (Pool cannot read PSUM and DMA cannot read PSUM, so a
# Pool lane would cost DVE the same feeder op as doing the exp on DVE.)
LANES = ("DADAADADAADADAADADAADAADADAADADA"
         "ADADAADADAADAADADAADADAADADAADAA")  # 39 A / 25 D per 64


def build_program():
    nc = bacc.Bacc()
    xt_d = nc.dram_tensor("xt", [D, S], F32R, kind="ExternalInput")
    wq_d = nc.dram_tensor("wq", [128, 4, 512], F32R, kind="ExternalInput")
    wk_d = nc.dram_tensor("wk", [128, 4, 512], F32R, kind="ExternalInput")
    wv_d = nc.dram_tensor("wv", [128, 4, 512], F32R, kind="ExternalInput")
    wo_d = nc.dram_tensor("wo", [64, 8, 512], F32R, kind="ExternalInput")
    bq_d = nc.dram_tensor("bq", [128, 4], F32, kind="ExternalInput")
    bk_d = nc.dram_tensor("bk", [128, 4], F32, kind="ExternalInput")
    bv_d = nc.dram_tensor("bv", [512], F32, kind="ExternalInput")
    bo_d = nc.dram_tensor("bo", [512], F32, kind="ExternalInput")
    out_d = nc.dram_tensor("out", [QCHUNK, D], F32, kind="ExternalOutput")

    with tile.TileContext(nc) as tc:
        with (
            tc.tile_pool(name="const", bufs=1) as cpool,
            tc.tile_pool(name="work", bufs=4) as wpool,
            tc.tile_pool(name="xtp", bufs=2) as xpool,
            tc.tile_pool(name="psc", bufs=3, space="PSUM") as pscpool,
            tc.tile_pool(name="pav", bufs=2, space="PSUM") as pavpool,
            tc.tile_pool(name="dr", bufs=2, space="DRAM") as dpool,
        ):
            wq_s = cpool.tile([128, 4, 512], F32R, tag="wq")
            wk_s = cpool.tile([128, 4, 512], F32R, tag="wk")
            wv_s = cpool.tile([128, 4, 512], F32R, tag="wv")
            wo_s = cpool.tile([64, 8, 512], F32R, tag="wo")
            bq_s = cpool.tile([128, 4], F32, tag="bq")
            bk_s = cpool.tile([128, 4], F32, tag="bk")
            bv_r = cpool.tile([128, 512], F32, tag="bvr")
            bo_r = cpool.tile([128, 512], F32, tag="bor")
            # only K weights up front; the rest stream in between chunk-0
            # tasks so the first projection starts as early as possible
            nc.sync.dma_start(wk_s[:], wk_d[:])
            nc.sync.dma_start(bk_s[:], bk_d[:])

            # fp8 operand tensors
            kT8 = cpool.tile([128, 4, S], FP8, tag="kT8")
            qT8 = cpool.tile([128, 4, QCHUNK], FP8, tag="qT8")
            vA = cpool.tile([128, S // 128, H, E + 2], FP8, tag="vA")
            yacc = cpool.tile([65, 16, 512], F32, tag="yacc")
            yT2 = cpool.tile([64, H, QCHUNK], F32R, tag="yT2")
            nc.vector.memset(vA[:, :, :, E], 1.0)
            nc.vector.memset(vA[:, :, :, E + 1], 0.0)

            def make_proj_tasks(ch):
                """Per-chunk projection split into psc-ring-sized tasks so it
                can interleave with attention blocks at fine grain."""
                state = {}

                def t_x():
                    xT = xpool.tile([128, 4, TCH], F32R, tag="xT")
                    for ds in range(4):
                        nc.sync.dma_start(
                            xT[:, ds, :],
                            xt_d[ds * 128:(ds + 1) * 128,
                                 ch * TCH:(ch + 1) * TCH])
                    state["xT"] = xT

                def t_k(ega):
                    def f():
                        xT = state["xT"]
                        pk = pscpool.tile([128, 2, 512], F32, tag="psc")
                        for i in range(2):
                            eg = ega * 2 + i
                            for ds in range(4):
                                nc.tensor.matmul(
                                    pk[:, i, :],
                                    wk_s[:, ds, eg * 128:(eg + 1) * 128],
                                    xT[:, ds, :], start=(ds == 0),
                                    stop=(ds == 3))
                            nc.scalar.activation(
                                kT8[:, eg, ch * TCH:(ch + 1) * TCH],
                                pk[:, i, :],
                                mybir.ActivationFunctionType.Identity,
                                bias=bk_s[:, eg:eg + 1], scale=1.0)
                    return f

                def t_v(tsa):
                    def f():
                        xT = state["xT"]
                        pv = pscpool.tile([128, 2, 512], F32, tag="psc")
                        for i in range(2):
                            ts = tsa * 2 + i
                            for ds in range(4):
                                nc.tensor.matmul(
                                    pv[:, i, :],
                                    xT[:, ds, ts * 128:(ts + 1) * 128],
                                    wv_s[:, ds, :], start=(ds == 0),
                                    stop=(ds == 3))
                            nc.vector.tensor_tensor(
                                out=vA[:, ch * 4 + ts, :, 0:E],
                                in0=pv[:, i, :].rearrange("p (h e) -> p h e",
                                                          h=H),
                                in1=bv_r[:].rearrange("p (h e) -> p h e", h=H),
                                op=ADD)
                    return f

                def t_q(ega):
                    def f():
                        xT = state["xT"]
                        pq = pscpool.tile([128, 2, 512], F32, tag="psc")
                        for i in range(2):
                            eg = ega * 2 + i
                            for ds in range(4):
                                nc.tensor.matmul(
                                    pq[:, i, :],
                                    wq_s[:, ds, eg * 128:(eg + 1) * 128],
                                    xT[:, ds, :], start=(ds == 0),
                                    stop=(ds == 3))
                            nc.vector.tensor_scalar(
                                qT8[:, eg, ch * TCH:(ch + 1) * TCH],
                                pq[:, i, :], bq_s[:, eg:eg + 1], None, ADD)
                    return f

                def first():
                    t_x()
                    t_k(0)()

                if ch < QCHUNK // TCH:
                    # queries first so attention can start early
                    tasks = [first, t_q(0), t_v(0), t_k(1), t_v(1), t_q(1)]
                else:
                    tasks = [first, t_v(0), t_k(1), t_v(1)]
                return tasks

            def emit_proj_chunk(ch):
                for t in make_proj_tasks(ch):
                    t()

            lane_ctr = [0]

            def emit_block(h, sc, pairs, is_group_a, interleave=None):
                a0 = 32 * (h % 4)
                g0 = 2 * (h // 4)
                n = len(pairs)
                pav = pavpool.tile([128, 512], F32, tag="pav")

                def emit_pv(item):
                    j, tp, ptv = item
                    nc.tensor.matmul(
                        pav[0:66, :], vA[:, 2 * tp:2 * tp + 2, h, :], ptv,
                        start=(j == 0), stop=(j == n - 1), perf_mode=DR)

                pend = []
                for j, tp in enumerate(pairs):
                    lane = LANES[lane_ctr[0] % len(LANES)]
                    lane_ctr[0] += 1
                    psc = pscpool.tile([128, 2, 512], F32, tag="psc")
                    for kt in (0, 1):
                        tt = 2 * tp + kt
                        nc.tensor.matmul(
                            psc[:, kt, :],
                            kT8[a0:a0 + 32, g0:g0 + 2, tt * 128:(tt + 1) * 128],
                            qT8[a0:a0 + 32, g0:g0 + 2, sc * 512:(sc + 1) * 512],
                            start=True, stop=True, perf_mode=DR,
                            tile_position=(a0, 0))
                    if lane == "A":
                        pt = wpool.tile([128, 2, 512], FP8, tag="ptA")
                        nc.scalar.activation(
                            pt[:], psc[:], mybir.ActivationFunctionType.Exp,
                            scale=0.125)
                        ptv = pt[:]
                    elif lane == "D":
                        pti = wpool.tile([128, 2, 512], I8, tag="ptD")
                        nc.vector.tensor_scalar(
                            pti[:], psc[:], A_SCH, B_SCH, MULT, ADD)
                        ptv = pti[:].bitcast(FP8)
                    else:  # lane P: DVE copies PSUM->SBUF bf16, Pool does sch
                        scf = wpool.tile([128, 2, 512], BF16, tag="scf")
                        nc.vector.tensor_copy(scf[:], psc[:])
                        pti = wpool.tile([128, 2, 512], I8, tag="ptP")
                        nc.gpsimd.tensor_scalar(
                            pti[:], scf[:], A_SCH, B_SCH, MULT, ADD)
                        ptv = pti[:].bitcast(FP8)
                    pend.append((j, tp, ptv))
                    if interleave:
                        interleave.pop(0)()
                    if len(pend) == 3:
                        emit_pv(pend.pop(0))
                for item in pend:
                    emit_pv(item)

                slot = sc * 8 + h
                if is_group_a:
                    nc.vector.tensor_copy(yacc[:, slot, :], pav[0:65, :])
                else:
                    tmp = wpool.tile([65, 512], F32, tag="tmp")
                    nc.vector.tensor_tensor(
                        out=tmp[:], in0=yacc[:, slot, :], in1=pav[0:65, :],
                        op=ADD)
                    rec = wpool.tile([1, 512], F32, tag="rec")
                    nc.vector.reciprocal(rec[:], tmp[64:65, :])
                    rec_d = dpool.tile([1, 512], F32, tag="recd")
                    nc.sync.dma_start(rec_d[:], rec[:])
                    rrep = wpool.tile([64, 512], F32, tag="rrep")
                    nc.sync.dma_start(rrep[:], rec_d[:].to_broadcast((64, 512)))
                    nc.gpsimd.tensor_tensor(
                        out=yT2[0:64, h, sc * 512:(sc + 1) * 512],
                        in0=tmp[0:64, :], in1=rrep[:], op=MULT)

            def phase3_task(sc, sta):
                def f():
                    po = pscpool.tile([128, 2, 512], F32, tag="psc")
                    for i in range(2):
                        st = sc * 4 + sta * 2 + i
                        for h in range(H):
                            nc.tensor.matmul(
                                po[:, i, :], yT2[0:64, h, st * 128:(st + 1) * 128],
                                wo_s[0:64, h, :], start=(h == 0), stop=(h == 7))
                        o_s = wpool.tile([128, 512], F32, tag="osb")
                        nc.vector.tensor_tensor(out=o_s[:, :], in0=po[:, i, :],
                                                in1=bo_r[:], op=ADD)
                        nc.sync.dma_start(out_d[st * 128:(st + 1) * 128, :],
                                          o_s[:])
                return f

            # ---- emission ----
            tasks0 = make_proj_tasks(0)
            tasks0[0]()  # x0 DMA + K egs 0,1
            nc.sync.dma_start(wq_s[:], wq_d[:])
            nc.sync.dma_start(bq_s[:], bq_d[:])
            tasks0[1]()  # Q egs 0,1
            nc.sync.dma_start(wv_s[:], wv_d[:])
            nc.sync.dma_start(bv_r[:], bv_d[:].unsqueeze(0).to_broadcast((128, 512)))
            for t in tasks0[2:]:
                t()
            emit_proj_chunk(1)
            c2 = make_proj_tasks(2)
            c3 = make_proj_tasks(3)
            inter = [c2[0], c2[1], c2[2], c3[0], c2[3], c3[1], c3[2], c3[3]]
            # chunks 4..7 queued as fine-grained tasks, 2 per block boundary
            proj_queue = []
            for ch in range(4, 8):
                proj_queue.extend(make_proj_tasks(ch))
            blocks = [(sc, h) for sc in (0, 1) for h in range(H)]
            for bi, (sc, h) in enumerate(blocks):
                emit_block(h, sc, GROUP_A, True,
                           interleave=inter if bi == 0 else None)
                if bi == 0:
                    nc.sync.dma_start(
                        bo_r[:], bo_d[:].unsqueeze(0).to_broadcast((128, 512)))
                    nc.sync.dma_start(wo_s[:], wo_d[:])
                for _ in range(2):
                    if proj_queue:
                        proj_queue.pop(0)()
            while proj_queue:
                proj_queue.pop(0)()
            # group B; spread each sc's output projection into the next sc's
            # blocks (final sc's at the end)
            pending_p3 = []
            for sc in (0, 1):
                for h in range(H):
                    emit_block(h, sc, GROUP_B, False)
                    if pending_p3:
                        pending_p3.pop(0)()
                pending_p3 = [phase3_task(sc, 0), phase3_task(sc, 1)]
            for t in pending_p3:
                t()
    nc.compile()
    return nc


_NC = None


def _pack_weights(Wq, bq, Wk, bk, Wv, bv, Wo, bo):
    s = lambda a: np.ascontiguousarray(np.asarray(a, np.float32))
    # e-permutation for DoubleRow plane layout: column c = eg*128+p of the
    # stationary maps to head h = p//32 + 4*(eg//2), e = 32*(eg%2) + p%32
    p = np.arange(128)
    eg = np.arange(4)
    hh = p[None, :] // 32 + 4 * (eg[:, None] // 2)     # [4,128]
    ee = 32 * (eg[:, None] % 2) + p[None, :] % 32      # [4,128]

    def pack_qk(W):
        t = np.asarray(W, np.float32)[hh, :, ee]       # [4,128,512(d)]
        t = t.transpose(2, 0, 1)                       # [d, eg, p]
        t = t.reshape(4, 128, 4, 128)                  # [ds, pd, eg, p]
        return s(t.transpose(1, 0, 2, 3).reshape(128, 4, 512))

    def pack_b(b):
        return s(np.asarray(b, np.float32)[hh, ee].T)  # [128,4]

    wq_p = pack_qk(Wq)
    wk_p = pack_qk(Wk)
    bq_p = pack_b(bq)
    bk_p = pack_b(bk)
    wv_p = s(np.transpose(Wv, (1, 0, 2)).reshape(D, 512).reshape(4, 128, 512)
             .transpose(1, 0, 2))
    wo_p = s(np.asarray(Wo, np.float32).reshape(8, 64, 512).transpose(1, 0, 2))
    bv_p = s(np.asarray(bv, np.float32).reshape(512))
    bo_p = s(np.asarray(bo, np.float32))
    return dict(wq=wq_p, wk=wk_p, wv=wv_p, wo=wo_p, bq=bq_p, bk=bk_p,
                bv=bv_p, bo=bo_p)


def kernel(x, Wq, bq, Wk, bk, Wv, bv, Wo, bo, **kw):
    global _NC
    x = np.asarray(x, np.float32)
    packed = _pack_weights(Wq, bq, Wk, bk, Wv, bv, Wo, bo)

    if _NC is None:
        _NC = build_program()

    in_maps = []
    for c in range(NCORES):
        b = c // 4
        q0 = (c % 4) * QCHUNK
        xb = np.roll(x[b], -q0, axis=0)  # queries at rows 0:1024
        m = {"xt": np.ascontiguousarray(xb.T)}
        m.update(packed)
        in_maps.append(m)
    res = run_bass_kernel_spmd(_NC, in_maps, core_ids=list(range(NCORES)))
    out = np.empty((B, S, D), np.float32)
    for c in range(NCORES):
        b = c // 4
        q0 = (c % 4) * QCHUNK
        out[b, q0:q0 + QCHUNK] = res.results[c]["out"]
    return out


# revision 24
# speedup vs baseline: 1.0395x; 1.0026x over previous
"""Multi-head attention Trainium2 kernel, 8-core SPMD (v2, fp8 DoubleRow).

Problem: x[2,4096,512], 8 heads of 64; per-head QKV proj, softmax(QK^T/8)V,
concat, output proj.

Sharding: sequence-parallel, no collectives. Core c handles batch b=c//4 and
query rows [1024*(c%4), ...+1024). Each core computes K/V for the full 4096-row
sequence of its batch; x is host-rolled so local query rows sit at 0:1024.

Speed structure:
- Q/K/V projections in f32r; K/Q converted to fp8-e4m3 with a host-side
  weight-column permutation so the PSUM->SBUF convert is partition-preserving
  and lands directly in the DoubleRow plane layout.
- Scores and PV use fp8-e4m3 DoubleRow matmuls (0.5 cyc/row): score stationary
  kT8[32,2,128] / moving qT8[32,2,512]; PV stationary vA[128,2,66] (a ones
  column provides the softmax denominator; a zero pad column satisfies the
  dual-fp8 Ldweights even-column ISA restriction) / moving pt[128,2,512].
- softmax exp is the wall; it is split across two engines by t-tile pair:
  lane A: ACT native Exp -> fp8 (scale=0.125 fused)
  lane D: DVE tensor_scalar computes e4m3 BITS of exp directly
          (Schraudolph: round(A*s+B) -> int8 -> bitcast fp8e4)
  (Pool/DMA cannot read PSUM, so no third lane is possible.)
- Attention is streamed in two t-groups (pairs 0-6, 7-15); K/V/Q projection of
  chunks 4..7 interleaves into group-A block boundaries at psc-ring-tile
  granularity; per-(h,sc) PV accumulates in one PSUM bank per group, group A
  drained to SBUF (yacc) and merged + normalized at group-B block end. PSUM:
  3x2-bank score/proj ring + 2 pav accumulators = 8 banks.
"""

import numpy as np

import concourse.bass as bass
from concourse import bacc
import concourse.mybir as mybir
import concourse.tile as tile
from concourse.bass_utils import run_bass_kernel_spmd

F32 = mybir.dt.float32
F32R = mybir.dt.float32r
BF16 = mybir.dt.bfloat16
FP8 = mybir.dt.float8e4
I8 = mybir.dt.int8
DR = mybir.MatmulPerfMode.DoubleRow
ADD = mybir.AluOpType.add
MULT = mybir.AluOpType.mult

B, S, D, H, E = 2, 4096, 512, 8, 64
NCORES = 8
QCHUNK = S // 4          # 1024 query rows per core
TCH = 512                # t-rows per projection chunk
NPAIR = S // 256         # 16 t-tile pairs
GROUP_A = list(range(8))        # chunks 0..3
GROUP_B = list(range(8, NPAIR)) # chunks 4..7

A_SCH = 1.442695041      # 8*log2(e)*0.125
B_SCH = 55.632           # calibrated: max rel err 7.2% per weight

# exp lane pattern, cycled over all 256 (h,sc,pair) tasks: A=ACT native exp,
# D=DVE Schraudolph. (Pool cannot read PSUM and DMA cannot read PSUM, so a
# Pool lane would cost DVE the same feeder op as doing the exp on DVE.)
LANES = ("DADAADADAADADAADADAADAADADAADADA"
         "ADADAADADAADAADADAADADAADADAADAA")  # 39 A / 25 D per 64


def build_program():
    nc = bacc.Bacc()
    xt_d = nc.dram_tensor("xt", [D, S], F32R, kind="ExternalInput")
    wq_d = nc.dram_tensor("wq", [128, 4, 512], F32R, kind="ExternalInput")
    wk_d = nc.dram_tensor("wk", [128, 4, 512], F32R, kind="ExternalInput")
    wv_d = nc.dram_tensor("wv", [128, 4, 512], F32R, kind="ExternalInput")
    wo_d = nc.dram_tensor("wo", [64, 8, 512], F32R, kind="ExternalInput")
    bq_d = nc.dram_tensor("bq", [128, 4], F32, kind="ExternalInput")
    bk_d = nc.dram_tensor("bk", [128, 4], F32, kind="ExternalInput")
    bv_d = nc.dram_tensor("bv", [512], F32, kind="ExternalInput")
    bo_d = nc.dram_tensor("bo", [512], F32, kind="ExternalInput")
    out_d = nc.dram_tensor("out", [QCHUNK, D], F32, kind="ExternalOutput")

    with tile.TileContext(nc) as tc:
        with (
            tc.tile_pool(name="const", bufs=1) as cpool,
            tc.tile_pool(name="work", bufs=4) as wpool,
            tc.tile_pool(name="xtp", bufs=2) as xpool,
            tc.tile_pool(name="psc", bufs=3, space="PSUM") as pscpool,
            tc.tile_pool(name="pav", bufs=2, space="PSUM") as pavpool,
            tc.tile_pool(name="dr", bufs=2, space="DRAM") as dpool,
        ):
            wq_s = cpool.tile([128, 4, 512], F32R, tag="wq")
            wk_s = cpool.tile([128, 4, 512], F32R, tag="wk")
            wv_s = cpool.tile([128, 4, 512], F32R, tag="wv")
            wo_s = cpool.tile([64, 8, 512], F32R, tag="wo")
            bq_s = cpool.tile([128, 4], F32, tag="bq")
            bk_s = cpool.tile([128, 4], F32, tag="bk")
            bv_r = cpool.tile([128, 512], F32, tag="bvr")
            bo_r = cpool.tile([128, 512], F32, tag="bor")
            # only K weights up front; the rest stream in between chunk-0
            # tasks so the first projection starts as early as possible
            nc.sync.dma_start(wk_s[:], wk_d[:])
            nc.sync.dma_start(bk_s[:], bk_d[:])

            # fp8 operand tensors
            kT8 = cpool.tile([128, 4, S], FP8, tag="kT8")
            qT8 = cpool.tile([128, 4, QCHUNK], FP8, tag="qT8")
            vA = cpool.tile([128, S // 128, H, E + 2], FP8, tag="vA")
            yacc = cpool.tile([65, 16, 512], F32, tag="yacc")
            yT2 = cpool.tile([64, H, QCHUNK], F32R, tag="yT2")
            nc.vector.memset(vA[:, :, :, E], 1.0)
            nc.vector.memset(vA[:, :, :, E + 1], 0.0)

            def make_proj_tasks(ch):
                """Per-chunk projection split into psc-ring-sized tasks so it
                can interleave with attention blocks at fine grain."""
                state = {}

                def t_x():
                    xT = xpool.tile([128, 4, TCH], F32R, tag="xT")
                    for ds in range(4):
                        nc.sync.dma_start(
                            xT[:, ds, :],
                            xt_d[ds * 128:(ds + 1) * 128,
                                 ch * TCH:(ch + 1) * TCH])
                    state["xT"] = xT

                def t_k(ega):
                    def f():
                        xT = state["xT"]
                        pk = pscpool.tile([128, 2, 512], F32, tag="psc")
                        for i in range(2):
                            eg = ega * 2 + i
                            for ds in range(4):
                                nc.tensor.matmul(
                                    pk[:, i, :],
                                    wk_s[:, ds, eg * 128:(eg + 1) * 128],
                                    xT[:, ds, :], start=(ds == 0),
                                    stop=(ds == 3))
                            nc.scalar.activation(
                                kT8[:, eg, ch * TCH:(ch + 1) * TCH],
                                pk[:, i, :],
                                mybir.ActivationFunctionType.Identity,
                                bias=bk_s[:, eg:eg + 1], scale=1.0)
                    return f

                def t_v(tsa):
                    def f():
                        xT = state["xT"]
                        pv = pscpool.tile([128, 2, 512], F32, tag="psc")
                        for i in range(2):
                            ts = tsa * 2 + i
                            for ds in range(4):
                                nc.tensor.matmul(
                                    pv[:, i, :],
                                    xT[:, ds, ts * 128:(ts + 1) * 128],
                                    wv_s[:, ds, :], start=(ds == 0),
                                    stop=(ds == 3))
                            nc.vector.tensor_tensor(
                                out=vA[:, ch * 4 + ts, :, 0:E],
                                in0=pv[:, i, :].rearrange("p (h e) -> p h e",
                                                          h=H),
                                in1=bv_r[:].rearrange("p (h e) -> p h e", h=H),
                                op=ADD)
                    return f

                def t_q(ega):
                    def f():
                        xT = state["xT"]
                        pq = pscpool.tile([128, 2, 512], F32, tag="psc")
                        for i in range(2):
                            eg = ega * 2 + i
                            for ds in range(4):
                                nc.tensor.matmul(
                                    pq[:, i, :],
                                    wq_s[:, ds, eg * 128:(eg + 1) * 128],
                                    xT[:, ds, :], start=(ds == 0),
                                    stop=(ds == 3))
                            nc.vector.tensor_scalar(
                                qT8[:, eg, ch * TCH:(ch + 1) * TCH],
                                pq[:, i, :], bq_s[:, eg:eg + 1], None, ADD)
                    return f

                def first():
                    t_x()
                    t_k(0)()

                if ch < QCHUNK // TCH:
                    # queries first so attention can start early
                    tasks = [first, t_q(0), t_v(0), t_k(1), t_v(1), t_q(1)]
                else:
                    tasks = [first, t_v(0), t_k(1), t_v(1)]
                return tasks

            def emit_proj_chunk(ch):
                for t in make_proj_tasks(ch):
                    t()

            lane_ctr = [0]

            def emit_block(h, sc, pairs, is_group_a, interleave=None):
                a0 = 32 * (h % 4)
                g0 = 2 * (h // 4)
                n = len(pairs)
                pav = pavpool.tile([128, 512], F32, tag="pav")

                def emit_pv(item):
                    j, tp, ptv = item
                    nc.tensor.matmul(
                        pav[0:66, :], vA[:, 2 * tp:2 * tp + 2, h, :], ptv,
                        start=(j == 0), stop=(j == n - 1), perf_mode=DR)

                pend = []
                for j, tp in enumerate(pairs):
                    lane = LANES[lane_ctr[0] % len(LANES)]
                    lane_ctr[0] += 1
                    psc = pscpool.tile([128, 2, 512], F32, tag="psc")
                    for kt in (0, 1):
                        tt = 2 * tp + kt
                        nc.tensor.matmul(
                            psc[:, kt, :],
                            kT8[a0:a0 + 32, g0:g0 + 2, tt * 128:(tt + 1) * 128],
                            qT8[a0:a0 + 32, g0:g0 + 2, sc * 512:(sc + 1) * 512],
                            start=True, stop=True, perf_mode=DR,
                            tile_position=(a0, 0))
                    if lane == "A":
                        pt = wpool.tile([128, 2, 512], FP8, tag="ptA")
                        nc.scalar.activation(
                            pt[:], psc[:], mybir.ActivationFunctionType.Exp,
                            scale=0.125)
                        ptv = pt[:]
                    elif lane == "D":
                        pti = wpool.tile([128, 2, 512], I8, tag="ptD")
                        nc.vector.tensor_scalar(
                            pti[:], psc[:], A_SCH, B_SCH, MULT, ADD)
                        ptv = pti[:].bitcast(FP8)
                    else:  # lane P: DVE copies PSUM->SBUF bf16, Pool does sch
                        scf = wpool.tile([128, 2, 512], BF16, tag="scf")
                        nc.vector.tensor_copy(scf[:], psc[:])
                        pti = wpool.tile([128, 2, 512], I8, tag="ptP")
                        nc.gpsimd.tensor_scalar(
                            pti[:], scf[:], A_SCH, B_SCH, MULT, ADD)
                        ptv = pti[:].bitcast(FP8)
                    pend.append((j, tp, ptv))
                    if interleave:
                        interleave.pop(0)()
                    if len(pend) == 3:
                        emit_pv(pend.pop(0))
                for item in pend:
                    emit_pv(item)

                slot = sc * 8 + h
                if is_group_a:
                    nc.vector.tensor_copy(yacc[:, slot, :], pav[0:65, :])
                else:
                    tmp = wpool.tile([65, 512], F32, tag="tmp")
                    nc.vector.tensor_tensor(
                        out=tmp[:], in0=yacc[:, slot, :], in1=pav[0:65, :],
                        op=ADD)
                    rec = wpool.tile([1, 512], F32, tag="rec")
                    nc.vector.reciprocal(rec[:], tmp[64:65, :])
                    rec_d = dpool.tile([1, 512], F32, tag="recd")
                    nc.sync.dma_start(rec_d[:], rec[:])
                    rrep = wpool.tile([64, 512], F32, tag="rrep")
                    nc.sync.dma_start(rrep[:], rec_d[:].to_broadcast((64, 512)))
                    nc.gpsimd.tensor_tensor(
                        out=yT2[0:64, h, sc * 512:(sc + 1) * 512],
                        in0=tmp[0:64, :], in1=rrep[:], op=MULT)

            def phase3_task(sc, sta):
                def f():
                    po = pscpool.tile([128, 2, 512], F32, tag="psc")
                    for i in range(2):
                        st = sc * 4 + sta * 2 + i
                        for h in range(H):
                            nc.tensor.matmul(
                                po[:, i, :], yT2[0:64, h, st * 128:(st + 1) * 128],
                                wo_s[0:64, h, :], start=(h == 0), stop=(h == 7))
                        o_s = wpool.tile([128, 512], F32, tag="osb")
                        nc.vector.tensor_tensor(out=o_s[:, :], in0=po[:, i, :],
                                                in1=bo_r[:], op=ADD)
                        nc.sync.dma_start(out_d[st * 128:(st + 1) * 128, :],
                                          o_s[:])
                return f

            # ---- emission ----
            tasks0 = make_proj_tasks(0)
            tasks0[0]()  # x0 DMA + K egs 0,1
            nc.sync.dma_start(wq_s[:], wq_d[:])
            nc.sync.dma_start(bq_s[:], bq_d[:])
            tasks0[1]()  # Q egs 0,1
            nc.sync.dma_start(wv_s[:], wv_d[:])
            nc.sync.dma_start(bv_r[:], bv_d[:].unsqueeze(0).to_broadcast((128, 512)))
            for t in tasks0[2:]:
                t()
            emit_proj_chunk(1)
            c2 = make_proj_tasks(2)
            c3 = make_proj_tasks(3)
            # chunks 4..7 queued as fine-grained tasks, 2 per block boundary
            proj_queue = []
            for ch in range(4, 8):
                proj_queue.extend(make_proj_tasks(ch))
            blocks = [(sc, h) for sc in (0, 1) for h in range(H)]
            # first two blocks run only pairs 0-3 (chunks 0-1) so attention
            # starts before chunks 2-3 project; their remaining pairs move to
            # their group-B blocks
            for bi, (sc, h) in enumerate(blocks):
                if bi == 0:
                    emit_block(h, sc, GROUP_A[:4], True, interleave=c2)
                elif bi == 1:
                    emit_block(h, sc, GROUP_A[:4], True, interleave=c3)
                else:
                    emit_block(h, sc, GROUP_A, True)
                if bi == 1:
                    nc.sync.dma_start(
                        bo_r[:], bo_d[:].unsqueeze(0).to_broadcast((128, 512)))
                    nc.sync.dma_start(wo_s[:], wo_d[:])
                for _ in range(2):
                    if proj_queue:
                        proj_queue.pop(0)()
            while proj_queue:
                proj_queue.pop(0)()
            # group B; spread each sc's output projection into the next sc's
            # blocks (final sc's at the end)
            pending_p3 = []
            for sc in (0, 1):
                for h in range(H):
                    bpairs = GROUP_B
                    if sc == 0 and h < 2:
                        bpairs = GROUP_A[4:] + GROUP_B
                    emit_block(h, sc, bpairs, False)
                    if pending_p3 and h % 4 == 1:
                        pending_p3.pop(0)()
                pending_p3 = [phase3_task(sc, 0), phase3_task(sc, 1)]
            for t in pending_p3:
                t()
    nc.compile()
    return nc


_NC = None


def _pack_weights(Wq, bq, Wk, bk, Wv, bv, Wo, bo):
    s = lambda a: np.ascontiguousarray(np.asarray(a, np.float32))
    # e-permutation for DoubleRow plane layout: column c = eg*128+p of the
    # stationary maps to head h = p//32 + 4*(eg//2), e = 32*(eg%2) + p%32
    p = np.arange(128)
    eg = np.arange(4)
    hh = p[None, :] // 32 + 4 * (eg[:, None] // 2)     # [4,128]
    ee = 32 * (eg[:, None] % 2) + p[None, :] % 32      # [4,128]

    def pack_qk(W):
        t = np.asarray(W, np.float32)[hh, :, ee]       # [4,128,512(d)]
        t = t.transpose(2, 0, 1)                       # [d, eg, p]
        t = t.reshape(4, 128, 4, 128)                  # [ds, pd, eg, p]
        return s(t.transpose(1, 0, 2, 3).reshape(128, 4, 512))

    def pack_b(b):
        return s(np.asarray(b, np.float32)[hh, ee].T)  # [128,4]

    wq_p = pack_qk(Wq)
    wk_p = pack_qk(Wk)
    bq_p = pack_b(bq)
    bk_p = pack_b(bk)
    wv_p = s(np.transpose(Wv, (1, 0, 2)).reshape(D, 512).reshape(4, 128, 512)
             .transpose(1, 0, 2))
    wo_p = s(np.asarray(Wo, np.float32).reshape(8, 64, 512).transpose(1, 0, 2))
    bv_p = s(np.asarray(bv, np.float32).reshape(512))
    bo_p = s(np.asarray(bo, np.float32))
    return dict(wq=wq_p, wk=wk_p, wv=wv_p, wo=wo_p, bq=bq_p, bk=bk_p,
                bv=bv_p, bo=bo_p)


def kernel(x, Wq, bq, Wk, bk, Wv, bv, Wo, bo, **kw):
    global _NC
    x = np.asarray(x, np.float32)
    packed = _pack_weights(Wq, bq, Wk, bk, Wv, bv, Wo, bo)

    if _NC is None:
        _NC = build_program()

    in_maps = []
    for c in range(NCORES):
        b = c // 4
        q0 = (c % 4) * QCHUNK
        xb = np.roll(x[b], -q0, axis=0)  # queries at rows 0:1024
        m = {"xt": np.ascontiguousarray(xb.T)}
        m.update(packed)
        in_maps.append(m)
    res = run_bass_kernel_spmd(_NC, in_maps, core_ids=list(range(NCORES)))
    out = np.empty((B, S, D), np.float32)
    for c in range(NCORES):
        b = c // 4
        q0 = (c % 4) * QCHUNK
        out[b, q0:q0 + QCHUNK] = res.results[c]["out"]
    return out


# revision 26
# speedup vs baseline: 1.0583x; 1.0181x over previous
"""Multi-head attention Trainium2 kernel, 8-core SPMD (v2, fp8 DoubleRow).

Problem: x[2,4096,512], 8 heads of 64; per-head QKV proj, softmax(QK^T/8)V,
concat, output proj.

Sharding: sequence-parallel, no collectives. Core c handles batch b=c//4 and
query rows [1024*(c%4), ...+1024). Each core computes K/V for the full 4096-row
sequence of its batch; x is host-rolled so local query rows sit at 0:1024.

Speed structure:
- Q/K/V projections in f32r; K/Q converted to fp8-e4m3 with a host-side
  weight-column permutation so the PSUM->SBUF convert is partition-preserving
  and lands directly in the DoubleRow plane layout.
- Scores and PV use fp8-e4m3 DoubleRow matmuls (0.5 cyc/row): score stationary
  kT8[32,2,128] / moving qT8[32,2,512]; PV stationary vA[128,2,66] (a ones
  column provides the softmax denominator; a zero pad column satisfies the
  dual-fp8 Ldweights even-column ISA restriction) / moving pt[128,2,512].
- softmax exp is the wall; it is split across two engines by t-tile pair:
  lane A: ACT native Exp -> fp8 (scale=0.125 fused)
  lane D: DVE tensor_scalar computes e4m3 BITS of exp directly
          (Schraudolph: round(A*s+B) -> int8 -> bitcast fp8e4)
  (Pool/DMA cannot read PSUM, so no third lane is possible.)
- Attention is streamed in two t-groups (pairs 0-7, 8-15; the first two
  blocks run pairs 0-3 only, handing 4-7 to their group-B blocks, so attention
  starts after just two projected chunks). K/V/Q projection interleaves into
  attention at psc-ring-tile granularity: chunks 2-3 inside the first two
  blocks, chunks 4-7 two tasks per block boundary; weight DMAs are staged just
  ahead of their first consumer. Per-(h,sc) PV accumulates in one PSUM bank
  per group; group A drains to SBUF (yacc) and is merged + normalized (DVE
  reciprocal + DMA-broadcast + Pool multiply) at group-B block end. PSUM
  budget: 3x2-bank score/projection ring + 2 pav accumulators = 8 banks.
"""

import numpy as np

import concourse.bass as bass
from concourse import bacc
import concourse.mybir as mybir
import concourse.tile as tile
from concourse.bass_utils import run_bass_kernel_spmd

F32 = mybir.dt.float32
F32R = mybir.dt.float32r
BF16 = mybir.dt.bfloat16
FP8 = mybir.dt.float8e4
I8 = mybir.dt.int8
DR = mybir.MatmulPerfMode.DoubleRow
ADD = mybir.AluOpType.add
MULT = mybir.AluOpType.mult

B, S, D, H, E = 2, 4096, 512, 8, 64
NCORES = 8
QCHUNK = S // 4          # 1024 query rows per core
TCH = 512                # t-rows per projection chunk
NPAIR = S // 256         # 16 t-tile pairs
GROUP_A = list(range(8))        # chunks 0..3
GROUP_B = list(range(8, NPAIR)) # chunks 4..7

A_SCH = 1.442695041      # 8*log2(e)*0.125
B_SCH = 55.632           # calibrated: max rel err 7.2% per weight

# exp lane pattern, cycled over all 256 (h,sc,pair) tasks: A=ACT native exp,
# D=DVE Schraudolph. (Pool cannot read PSUM and DMA cannot read PSUM, so a
# Pool lane would cost DVE the same feeder op as doing the exp on DVE.)
LANES = ("DADAADADAADADAADADAADAADADAADADA"
         "ADADAADADAADAADADAADADAADADAADAA")  # 39 A / 25 D per 64


def build_program():
    nc = bacc.Bacc()
    xt_d = nc.dram_tensor("xt", [D, S], BF16, kind="ExternalInput")
    wq_d = nc.dram_tensor("wq", [128, 4, 512], BF16, kind="ExternalInput")
    wk_d = nc.dram_tensor("wk", [128, 4, 512], BF16, kind="ExternalInput")
    wv_d = nc.dram_tensor("wv", [128, 4, 512], BF16, kind="ExternalInput")
    wo_d = nc.dram_tensor("wo", [64, 8, 512], F32R, kind="ExternalInput")
    bq_d = nc.dram_tensor("bq", [128, 4], F32, kind="ExternalInput")
    bk_d = nc.dram_tensor("bk", [128, 4], F32, kind="ExternalInput")
    bv_d = nc.dram_tensor("bv", [512], F32, kind="ExternalInput")
    bo_d = nc.dram_tensor("bo", [512], F32, kind="ExternalInput")
    out_d = nc.dram_tensor("out", [QCHUNK, D], F32, kind="ExternalOutput")

    with tile.TileContext(nc) as tc:
        with (
            tc.tile_pool(name="const", bufs=1) as cpool,
            tc.tile_pool(name="work", bufs=4) as wpool,
            tc.tile_pool(name="xtp", bufs=3) as xpool,
            tc.tile_pool(name="psc", bufs=3, space="PSUM") as pscpool,
            tc.tile_pool(name="pav", bufs=2, space="PSUM") as pavpool,
            tc.tile_pool(name="dr", bufs=2, space="DRAM") as dpool,
        ):
            wq_s = cpool.tile([128, 4, 512], BF16, tag="wq")
            wk_s = cpool.tile([128, 4, 512], BF16, tag="wk")
            wv_s = cpool.tile([128, 4, 512], BF16, tag="wv")
            wo_s = cpool.tile([64, 8, 512], F32R, tag="wo")
            bq_s = cpool.tile([128, 4], F32, tag="bq")
            bk_s = cpool.tile([128, 4], F32, tag="bk")
            bv_r = cpool.tile([128, 512], F32, tag="bvr")
            bo_r = cpool.tile([128, 512], F32, tag="bor")
            # only K weights up front; the rest stream in between chunk-0
            # tasks so the first projection starts as early as possible
            nc.sync.dma_start(wk_s[:], wk_d[:])
            nc.sync.dma_start(bk_s[:], bk_d[:])

            # fp8 operand tensors
            kT8 = cpool.tile([128, 4, S], FP8, tag="kT8")
            qT8 = cpool.tile([128, 4, QCHUNK], FP8, tag="qT8")
            vA = cpool.tile([128, S // 128, H, E + 2], FP8, tag="vA")
            yacc = cpool.tile([65, 16, 512], F32, tag="yacc")
            yT2 = cpool.tile([64, H, QCHUNK], F32R, tag="yT2")
            nc.vector.memset(vA[:, :, :, E], 1.0)
            nc.vector.memset(vA[:, :, :, E + 1], 0.0)

            def make_proj_tasks(ch):
                """Per-chunk projection split into psc-ring-sized tasks so it
                can interleave with attention blocks at fine grain."""
                state = {}

                def t_x():
                    xT = xpool.tile([128, 4, TCH], BF16, tag="xT")
                    for ds in range(4):
                        nc.sync.dma_start(
                            xT[:, ds, :],
                            xt_d[ds * 128:(ds + 1) * 128,
                                 ch * TCH:(ch + 1) * TCH])
                    state["xT"] = xT

                def t_k(ega):
                    def f():
                        xT = state["xT"]
                        pk = pscpool.tile([128, 2, 512], F32, tag="psc")
                        for i in range(2):
                            eg = ega * 2 + i
                            for ds in range(4):
                                nc.tensor.matmul(
                                    pk[:, i, :],
                                    wk_s[:, ds, eg * 128:(eg + 1) * 128],
                                    xT[:, ds, :], start=(ds == 0),
                                    stop=(ds == 3))
                            nc.scalar.activation(
                                kT8[:, eg, ch * TCH:(ch + 1) * TCH],
                                pk[:, i, :],
                                mybir.ActivationFunctionType.Identity,
                                bias=bk_s[:, eg:eg + 1], scale=1.0)
                    return f

                def t_v(tsa):
                    def f():
                        xT = state["xT"]
                        pv = pscpool.tile([128, 2, 512], F32, tag="psc")
                        for i in range(2):
                            ts = tsa * 2 + i
                            for ds in range(4):
                                nc.tensor.matmul(
                                    pv[:, i, :],
                                    xT[:, ds, ts * 128:(ts + 1) * 128],
                                    wv_s[:, ds, :], start=(ds == 0),
                                    stop=(ds == 3))
                            nc.vector.tensor_tensor(
                                out=vA[:, ch * 4 + ts, :, 0:E],
                                in0=pv[:, i, :].rearrange("p (h e) -> p h e",
                                                          h=H),
                                in1=bv_r[:].rearrange("p (h e) -> p h e", h=H),
                                op=ADD)
                    return f

                def t_q(ega):
                    def f():
                        xT = state["xT"]
                        pq = pscpool.tile([128, 2, 512], F32, tag="psc")
                        for i in range(2):
                            eg = ega * 2 + i
                            for ds in range(4):
                                nc.tensor.matmul(
                                    pq[:, i, :],
                                    wq_s[:, ds, eg * 128:(eg + 1) * 128],
                                    xT[:, ds, :], start=(ds == 0),
                                    stop=(ds == 3))
                            nc.vector.tensor_scalar(
                                qT8[:, eg, ch * TCH:(ch + 1) * TCH],
                                pq[:, i, :], bq_s[:, eg:eg + 1], None, ADD)
                    return f

                def first():
                    t_x()
                    t_k(0)()

                if ch < QCHUNK // TCH:
                    # queries first so attention can start early
                    tasks = [first, t_q(0), t_v(0), t_k(1), t_v(1), t_q(1)]
                else:
                    tasks = [first, t_v(0), t_k(1), t_v(1)]
                return tasks

            def emit_proj_chunk(ch):
                for t in make_proj_tasks(ch):
                    t()

            lane_ctr = [0]

            def emit_block(h, sc, pairs, is_group_a, interleave=None):
                a0 = 32 * (h % 4)
                g0 = 2 * (h // 4)
                n = len(pairs)
                pav = pavpool.tile([128, 512], F32, tag="pav")

                def emit_pv(item):
                    j, tp, ptv = item
                    nc.tensor.matmul(
                        pav[0:66, :], vA[:, 2 * tp:2 * tp + 2, h, :], ptv,
                        start=(j == 0), stop=(j == n - 1), perf_mode=DR)

                pend = []
                for j, tp in enumerate(pairs):
                    lane = LANES[lane_ctr[0] % len(LANES)]
                    lane_ctr[0] += 1
                    psc = pscpool.tile([128, 2, 512], F32, tag="psc")
                    for kt in (0, 1):
                        tt = 2 * tp + kt
                        nc.tensor.matmul(
                            psc[:, kt, :],
                            kT8[a0:a0 + 32, g0:g0 + 2, tt * 128:(tt + 1) * 128],
                            qT8[a0:a0 + 32, g0:g0 + 2, sc * 512:(sc + 1) * 512],
                            start=True, stop=True, perf_mode=DR,
                            tile_position=(a0, 0))
                    if lane == "A":
                        pt = wpool.tile([128, 2, 512], FP8, tag="ptA")
                        nc.scalar.activation(
                            pt[:], psc[:], mybir.ActivationFunctionType.Exp,
                            scale=0.125)
                        ptv = pt[:]
                    elif lane == "D":
                        pti = wpool.tile([128, 2, 512], I8, tag="ptD")
                        nc.vector.tensor_scalar(
                            pti[:], psc[:], A_SCH, B_SCH, MULT, ADD)
                        ptv = pti[:].bitcast(FP8)
                    else:  # lane P: DVE copies PSUM->SBUF bf16, Pool does sch
                        scf = wpool.tile([128, 2, 512], BF16, tag="scf")
                        nc.vector.tensor_copy(scf[:], psc[:])
                        pti = wpool.tile([128, 2, 512], I8, tag="ptP")
                        nc.gpsimd.tensor_scalar(
                            pti[:], scf[:], A_SCH, B_SCH, MULT, ADD)
                        ptv = pti[:].bitcast(FP8)
                    pend.append((j, tp, ptv))
                    if interleave:
                        interleave.pop(0)()
                    if len(pend) == 3:
                        emit_pv(pend.pop(0))
                for item in pend:
                    emit_pv(item)

                slot = sc * 8 + h
                if is_group_a:
                    nc.vector.tensor_copy(yacc[:, slot, :], pav[0:65, :])
                else:
                    tmp = wpool.tile([65, 512], F32, tag="tmp")
                    nc.vector.tensor_tensor(
                        out=tmp[:], in0=yacc[:, slot, :], in1=pav[0:65, :],
                        op=ADD)
                    rec = wpool.tile([1, 512], F32, tag="rec")
                    nc.vector.reciprocal(rec[:], tmp[64:65, :])
                    rec_d = dpool.tile([1, 512], F32, tag="recd")
                    nc.sync.dma_start(rec_d[:], rec[:])
                    rrep = wpool.tile([64, 512], F32, tag="rrep")
                    nc.sync.dma_start(rrep[:], rec_d[:].to_broadcast((64, 512)))
                    nc.gpsimd.tensor_tensor(
                        out=yT2[0:64, h, sc * 512:(sc + 1) * 512],
                        in0=tmp[0:64, :], in1=rrep[:], op=MULT)

            def phase3_task(sc, sta):
                def f():
                    po = pscpool.tile([128, 2, 512], F32, tag="psc")
                    for i in range(2):
                        st = sc * 4 + sta * 2 + i
                        for h in range(H):
                            nc.tensor.matmul(
                                po[:, i, :], yT2[0:64, h, st * 128:(st + 1) * 128],
                                wo_s[0:64, h, :], start=(h == 0), stop=(h == 7))
                        o_s = wpool.tile([128, 512], F32, tag="osb")
                        nc.vector.tensor_tensor(out=o_s[:, :], in0=po[:, i, :],
                                                in1=bo_r[:], op=ADD)
                        nc.sync.dma_start(out_d[st * 128:(st + 1) * 128, :],
                                          o_s[:])
                return f

            # ---- emission ----
            tasks0 = make_proj_tasks(0)
            tasks0[0]()  # x0 DMA + K egs 0,1
            nc.sync.dma_start(wq_s[:], wq_d[:])
            nc.sync.dma_start(bq_s[:], bq_d[:])
            tasks0[1]()  # Q egs 0,1
            nc.sync.dma_start(wv_s[:], wv_d[:])
            nc.sync.dma_start(bv_r[:], bv_d[:].unsqueeze(0).to_broadcast((128, 512)))
            for t in tasks0[2:]:
                t()
            emit_proj_chunk(1)
            c2 = make_proj_tasks(2)
            c3 = make_proj_tasks(3)
            # chunks 4..7 queued as fine-grained tasks, 2 per block boundary
            proj_queue = []
            for ch in range(4, 8):
                proj_queue.extend(make_proj_tasks(ch))
            blocks = [(sc, h) for sc in (0, 1) for h in range(H)]
            # first two blocks run only pairs 0-3 (chunks 0-1) so attention
            # starts before chunks 2-3 project; their remaining pairs move to
            # their group-B blocks
            for bi, (sc, h) in enumerate(blocks):
                if bi == 0:
                    emit_block(h, sc, GROUP_A[:4], True, interleave=c2)
                elif bi == 1:
                    emit_block(h, sc, GROUP_A[:4], True, interleave=c3)
                else:
                    emit_block(h, sc, GROUP_A, True)
                if bi == 1:
                    nc.sync.dma_start(
                        bo_r[:], bo_d[:].unsqueeze(0).to_broadcast((128, 512)))
                    nc.sync.dma_start(wo_s[:], wo_d[:])
                for _ in range(2):
                    if proj_queue:
                        proj_queue.pop(0)()
            while proj_queue:
                proj_queue.pop(0)()
            # group B; spread each sc's output projection into the next sc's
            # blocks (final sc's at the end)
            pending_p3 = []
            for sc in (0, 1):
                for h in range(H):
                    bpairs = GROUP_B
                    if sc == 0 and h < 2:
                        bpairs = GROUP_A[4:] + GROUP_B
                    emit_block(h, sc, bpairs, False)
                    if pending_p3 and h % 4 == 1:
                        pending_p3.pop(0)()
                pending_p3 = [phase3_task(sc, 0), phase3_task(sc, 1)]
            for t in pending_p3:
                t()
    nc.compile()
    return nc


_NC = None


def _pack_weights(Wq, bq, Wk, bk, Wv, bv, Wo, bo):
    import ml_dtypes
    s = lambda a: np.ascontiguousarray(np.asarray(a, np.float32))
    sb = lambda a: np.ascontiguousarray(
        np.asarray(a, np.float32).astype(ml_dtypes.bfloat16))
    # e-permutation for DoubleRow plane layout: column c = eg*128+p of the
    # stationary maps to head h = p//32 + 4*(eg//2), e = 32*(eg%2) + p%32
    p = np.arange(128)
    eg = np.arange(4)
    hh = p[None, :] // 32 + 4 * (eg[:, None] // 2)     # [4,128]
    ee = 32 * (eg[:, None] % 2) + p[None, :] % 32      # [4,128]

    def pack_qk(W):
        t = np.asarray(W, np.float32)[hh, :, ee]       # [4,128,512(d)]
        t = t.transpose(2, 0, 1)                       # [d, eg, p]
        t = t.reshape(4, 128, 4, 128)                  # [ds, pd, eg, p]
        return sb(t.transpose(1, 0, 2, 3).reshape(128, 4, 512))

    def pack_b(b):
        return s(np.asarray(b, np.float32)[hh, ee].T)  # [128,4]

    wq_p = pack_qk(Wq)
    wk_p = pack_qk(Wk)
    bq_p = pack_b(bq)
    bk_p = pack_b(bk)
    wv_p = sb(np.transpose(Wv, (1, 0, 2)).reshape(D, 512).reshape(4, 128, 512)
              .transpose(1, 0, 2))
    wo_p = s(np.asarray(Wo, np.float32).reshape(8, 64, 512).transpose(1, 0, 2))
    bv_p = s(np.asarray(bv, np.float32).reshape(512))
    bo_p = s(np.asarray(bo, np.float32))
    return dict(wq=wq_p, wk=wk_p, wv=wv_p, wo=wo_p, bq=bq_p, bk=bk_p,
                bv=bv_p, bo=bo_p)


def kernel(x, Wq, bq, Wk, bk, Wv, bv, Wo, bo, **kw):
    global _NC
    x = np.asarray(x, np.float32)
    packed = _pack_weights(Wq, bq, Wk, bk, Wv, bv, Wo, bo)

    if _NC is None:
        _NC = build_program()

    in_maps = []
    for c in range(NCORES):
        b = c // 4
        q0 = (c % 4) * QCHUNK
        xb = np.roll(x[b], -q0, axis=0)  # queries at rows 0:1024
        import ml_dtypes
        m = {"xt": np.ascontiguousarray(xb.T.astype(ml_dtypes.bfloat16))}
        m.update(packed)
        in_maps.append(m)
    res = run_bass_kernel_spmd(_NC, in_maps, core_ids=list(range(NCORES)))
    out = np.empty((B, S, D), np.float32)
    for c in range(NCORES):
        b = c // 4
        q0 = (c % 4) * QCHUNK
        out[b, q0:q0 + QCHUNK] = res.results[c]["out"]
    return out


# revision 30
# speedup vs baseline: 1.0604x; 1.0020x over previous
"""Multi-head attention Trainium2 kernel, 8-core SPMD (v2, fp8 DoubleRow).

Problem: x[2,4096,512], 8 heads of 64; per-head QKV proj, softmax(QK^T/8)V,
concat, output proj.

Sharding: sequence-parallel, no collectives. Core c handles batch b=c//4 and
query rows [1024*(c%4), ...+1024). Each core computes K/V for the full 4096-row
sequence of its batch; x is host-rolled so local query rows sit at 0:1024.

Speed structure:
- Q/K/V projections in f32r; K/Q converted to fp8-e4m3 with a host-side
  weight-column permutation so the PSUM->SBUF convert is partition-preserving
  and lands directly in the DoubleRow plane layout.
- Scores and PV use fp8-e4m3 DoubleRow matmuls (0.5 cyc/row): score stationary
  kT8[32,2,128] / moving qT8[32,2,512]; PV stationary vA[128,2,66] (a ones
  column provides the softmax denominator; a zero pad column satisfies the
  dual-fp8 Ldweights even-column ISA restriction) / moving pt[128,2,512].
- softmax exp is the wall; it is split across two engines by t-tile pair:
  lane A: ACT native Exp -> fp8 (scale=0.125 fused)
  lane D: DVE tensor_scalar computes e4m3 BITS of exp directly
          (Schraudolph: round(A*s+B) -> int8 -> bitcast fp8e4)
  (Pool/DMA cannot read PSUM, so no third lane is possible.)
- Attention is streamed in two t-groups (pairs 0-7, 8-15; the first two
  blocks run pairs 0-3 only, handing 4-7 to their group-B blocks, so attention
  starts after just two projected chunks). K/V/Q projection interleaves into
  attention at psc-ring-tile granularity: chunks 2-3 inside the first two
  blocks, chunks 4-7 two tasks per block boundary; weight DMAs are staged just
  ahead of their first consumer. Per-(h,sc) PV accumulates in one PSUM bank
  per group; group A drains to SBUF (yacc) and is merged + normalized (DVE
  reciprocal + DMA-broadcast + Pool multiply) at group-B block end. PSUM
  budget: 3x2-bank score/projection ring + 2 pav accumulators = 8 banks.
"""

import numpy as np

import concourse.bass as bass
from concourse import bacc
import concourse.mybir as mybir
import concourse.tile as tile
from concourse.bass_utils import run_bass_kernel_spmd

F32 = mybir.dt.float32
F32R = mybir.dt.float32r
BF16 = mybir.dt.bfloat16
FP8 = mybir.dt.float8e4
I8 = mybir.dt.int8
DR = mybir.MatmulPerfMode.DoubleRow
ADD = mybir.AluOpType.add
MULT = mybir.AluOpType.mult

B, S, D, H, E = 2, 4096, 512, 8, 64
NCORES = 8
QCHUNK = S // 4          # 1024 query rows per core
TCH = 512                # t-rows per projection chunk
NPAIR = S // 256         # 16 t-tile pairs
GROUP_A = list(range(8))        # chunks 0..3
GROUP_B = list(range(8, NPAIR)) # chunks 4..7

A_SCH = 1.442695041      # 8*log2(e)*0.125
B_SCH = 55.632           # calibrated: max rel err 7.2% per weight

# exp lane pattern, cycled over all 256 (h,sc,pair) tasks: A=ACT native exp,
# D=DVE Schraudolph. (Pool cannot read PSUM and DMA cannot read PSUM, so a
# Pool lane would cost DVE the same feeder op as doing the exp on DVE.)
LANES = ("DADAADADAADADAADADAADAADADAADADA"
         "ADADAADADAADAADADAADADAADADAADAA")  # 39 A / 25 D per 64


def build_program():
    nc = bacc.Bacc()
    xt_d = nc.dram_tensor("xt", [D, S], BF16, kind="ExternalInput")
    wq_d = nc.dram_tensor("wq", [128, 4, 512], BF16, kind="ExternalInput")
    wk_d = nc.dram_tensor("wk", [128, 4, 512], BF16, kind="ExternalInput")
    wv_d = nc.dram_tensor("wv", [128, 4, 512], BF16, kind="ExternalInput")
    wo_d = nc.dram_tensor("wo", [64, 8, 512], F32R, kind="ExternalInput")
    bq_d = nc.dram_tensor("bq", [128, 4], F32, kind="ExternalInput")
    bk_d = nc.dram_tensor("bk", [128, 4], F32, kind="ExternalInput")
    bv_d = nc.dram_tensor("bv", [512], F32, kind="ExternalInput")
    bo_d = nc.dram_tensor("bo", [512], F32, kind="ExternalInput")
    out_d = nc.dram_tensor("out", [QCHUNK, D], F32, kind="ExternalOutput")

    with tile.TileContext(nc) as tc:
        with (
            tc.tile_pool(name="const", bufs=1) as cpool,
            tc.tile_pool(name="work", bufs=4) as wpool,
            tc.tile_pool(name="xtp", bufs=3) as xpool,
            tc.tile_pool(name="psc", bufs=3, space="PSUM") as pscpool,
            tc.tile_pool(name="pav", bufs=2, space="PSUM") as pavpool,
            tc.tile_pool(name="dr", bufs=2, space="DRAM") as dpool,
        ):
            wq_s = cpool.tile([128, 4, 512], BF16, tag="wq")
            wk_s = cpool.tile([128, 4, 512], BF16, tag="wk")
            wv_s = cpool.tile([128, 4, 512], BF16, tag="wv")
            wo_s = cpool.tile([64, 8, 512], F32R, tag="wo")
            bq_s = cpool.tile([128, 4], F32, tag="bq")
            bk_s = cpool.tile([128, 4], F32, tag="bk")
            bv_r = cpool.tile([128, 512], F32, tag="bvr")
            bo_r = cpool.tile([128, 512], F32, tag="bor")
            # only K weights up front; the rest stream in between chunk-0
            # tasks so the first projection starts as early as possible
            nc.sync.dma_start(wk_s[:], wk_d[:])
            nc.sync.dma_start(bk_s[:], bk_d[:])

            # fp8 operand tensors
            kT8 = cpool.tile([128, 4, S], FP8, tag="kT8")
            qT8 = cpool.tile([128, 4, QCHUNK], FP8, tag="qT8")
            vA = cpool.tile([128, S // 128, H, E + 2], FP8, tag="vA")
            yacc = cpool.tile([65, 16, 512], F32, tag="yacc")
            yT2 = cpool.tile([64, H, QCHUNK], F32R, tag="yT2")
            nc.vector.memset(vA[:, :, :, E], 1.0)
            nc.vector.memset(vA[:, :, :, E + 1], 0.0)

            def make_proj_tasks(ch):
                """Per-chunk projection split into psc-ring-sized tasks so it
                can interleave with attention blocks at fine grain."""
                state = {}

                def t_x():
                    xT = xpool.tile([128, 4, TCH], BF16, tag="xT")
                    for ds in range(4):
                        nc.sync.dma_start(
                            xT[:, ds, :],
                            xt_d[ds * 128:(ds + 1) * 128,
                                 ch * TCH:(ch + 1) * TCH])
                    state["xT"] = xT

                def t_k(ega):
                    def f():
                        xT = state["xT"]
                        pk = pscpool.tile([128, 2, 512], F32, tag="psc")
                        for i in range(2):
                            eg = ega * 2 + i
                            for ds in range(4):
                                nc.tensor.matmul(
                                    pk[:, i, :],
                                    wk_s[:, ds, eg * 128:(eg + 1) * 128],
                                    xT[:, ds, :], start=(ds == 0),
                                    stop=(ds == 3))
                            nc.scalar.activation(
                                kT8[:, eg, ch * TCH:(ch + 1) * TCH],
                                pk[:, i, :],
                                mybir.ActivationFunctionType.Identity,
                                bias=bk_s[:, eg:eg + 1], scale=1.0)
                    return f

                def t_v(tsa):
                    def f():
                        xT = state["xT"]
                        pv = pscpool.tile([128, 2, 512], F32, tag="psc")
                        for i in range(2):
                            ts = tsa * 2 + i
                            for ds in range(4):
                                nc.tensor.matmul(
                                    pv[:, i, :],
                                    xT[:, ds, ts * 128:(ts + 1) * 128],
                                    wv_s[:, ds, :], start=(ds == 0),
                                    stop=(ds == 3))
                            nc.vector.tensor_tensor(
                                out=vA[:, ch * 4 + ts, :, 0:E],
                                in0=pv[:, i, :].rearrange("p (h e) -> p h e",
                                                          h=H),
                                in1=bv_r[:].rearrange("p (h e) -> p h e", h=H),
                                op=ADD)
                    return f

                def t_q(ega):
                    def f():
                        xT = state["xT"]
                        pq = pscpool.tile([128, 2, 512], F32, tag="psc")
                        for i in range(2):
                            eg = ega * 2 + i
                            for ds in range(4):
                                nc.tensor.matmul(
                                    pq[:, i, :],
                                    wq_s[:, ds, eg * 128:(eg + 1) * 128],
                                    xT[:, ds, :], start=(ds == 0),
                                    stop=(ds == 3))
                            nc.vector.tensor_scalar(
                                qT8[:, eg, ch * TCH:(ch + 1) * TCH],
                                pq[:, i, :], bq_s[:, eg:eg + 1], None, ADD)
                    return f

                def first():
                    t_x()
                    t_k(0)()

                if ch < QCHUNK // TCH:
                    # queries first so attention can start early
                    tasks = [first, t_q(0), t_v(0), t_k(1), t_v(1), t_q(1)]
                else:
                    tasks = [first, t_v(0), t_k(1), t_v(1)]
                return tasks

            def emit_proj_chunk(ch):
                for t in make_proj_tasks(ch):
                    t()

            lane_ctr = [0]

            def emit_block(h, sc, pairs, is_group_a, interleave=None):
                a0 = 32 * (h % 4)
                g0 = 2 * (h // 4)
                n = len(pairs)
                pav = pavpool.tile([128, 512], F32, tag="pav")

                def emit_pv(item):
                    j, tp, ptv = item
                    nc.tensor.matmul(
                        pav[0:66, :], vA[:, 2 * tp:2 * tp + 2, h, :], ptv,
                        start=(j == 0), stop=(j == n - 1), perf_mode=DR)

                pend = []
                for j, tp in enumerate(pairs):
                    lane = LANES[lane_ctr[0] % len(LANES)]
                    lane_ctr[0] += 1
                    psc = pscpool.tile([128, 2, 512], F32, tag="psc")
                    for kt in (0, 1):
                        tt = 2 * tp + kt
                        nc.tensor.matmul(
                            psc[:, kt, :],
                            kT8[a0:a0 + 32, g0:g0 + 2, tt * 128:(tt + 1) * 128],
                            qT8[a0:a0 + 32, g0:g0 + 2, sc * 512:(sc + 1) * 512],
                            start=True, stop=True, perf_mode=DR,
                            tile_position=(a0, 0))
                    if lane == "A":
                        pt = wpool.tile([128, 2, 512], FP8, tag="ptA")
                        nc.scalar.activation(
                            pt[:], psc[:], mybir.ActivationFunctionType.Exp,
                            scale=0.125)
                        ptv = pt[:]
                    elif lane == "D":
                        pti = wpool.tile([128, 2, 512], I8, tag="ptD")
                        nc.vector.tensor_scalar(
                            pti[:], psc[:], A_SCH, B_SCH, MULT, ADD)
                        ptv = pti[:].bitcast(FP8)
                    else:  # lane P: DVE copies PSUM->SBUF bf16, Pool does sch
                        scf = wpool.tile([128, 2, 512], BF16, tag="scf")
                        nc.vector.tensor_copy(scf[:], psc[:])
                        pti = wpool.tile([128, 2, 512], I8, tag="ptP")
                        nc.gpsimd.tensor_scalar(
                            pti[:], scf[:], A_SCH, B_SCH, MULT, ADD)
                        ptv = pti[:].bitcast(FP8)
                    pend.append((j, tp, ptv))
                    if interleave:
                        interleave.pop(0)()
                    if len(pend) == 3:
                        emit_pv(pend.pop(0))
                for item in pend:
                    emit_pv(item)

                slot = sc * 8 + h
                if is_group_a:
                    nc.vector.tensor_copy(yacc[:, slot, :], pav[0:65, :])
                else:
                    tmp = wpool.tile([65, 512], F32, tag="tmp")
                    nc.vector.tensor_tensor(
                        out=tmp[:], in0=yacc[:, slot, :], in1=pav[0:65, :],
                        op=ADD)
                    rec = wpool.tile([1, 512], F32, tag="rec")
                    nc.vector.reciprocal(rec[:], tmp[64:65, :])
                    rec_d = dpool.tile([1, 512], F32, tag="recd")
                    nc.sync.dma_start(rec_d[:], rec[:])
                    rrep = wpool.tile([64, 512], F32, tag="rrep")
                    nc.sync.dma_start(rrep[:], rec_d[:].to_broadcast((64, 512)))
                    nc.gpsimd.tensor_tensor(
                        out=yT2[0:64, h, sc * 512:(sc + 1) * 512],
                        in0=tmp[0:64, :], in1=rrep[:], op=MULT)

            def phase3_task(sc, sta):
                def f():
                    po = pscpool.tile([128, 2, 512], F32, tag="psc")
                    for i in range(2):
                        st = sc * 4 + sta * 2 + i
                        for h in range(H):
                            nc.tensor.matmul(
                                po[:, i, :], yT2[0:64, h, st * 128:(st + 1) * 128],
                                wo_s[0:64, h, :], start=(h == 0), stop=(h == 7))
                        o_s = wpool.tile([128, 512], F32, tag="osb")
                        nc.vector.tensor_tensor(out=o_s[:, :], in0=po[:, i, :],
                                                in1=bo_r[:], op=ADD)
                        nc.sync.dma_start(out_d[st * 128:(st + 1) * 128, :],
                                          o_s[:])
                return f

            # ---- emission ----
            tasks0 = make_proj_tasks(0)
            tasks0[0]()  # x0 DMA + K egs 0,1
            nc.sync.dma_start(wq_s[:], wq_d[:])
            nc.sync.dma_start(bq_s[:], bq_d[:])
            tasks0[1]()  # Q egs 0,1
            nc.sync.dma_start(wv_s[:], wv_d[:])
            nc.sync.dma_start(bv_r[:], bv_d[:].unsqueeze(0).to_broadcast((128, 512)))
            for t in tasks0[2:]:
                t()
            emit_proj_chunk(1)
            c2 = make_proj_tasks(2)
            c3 = make_proj_tasks(3)
            # chunks 4..7 queued as fine-grained tasks, 2 per block boundary
            proj_queue = []
            for ch in range(4, 8):
                proj_queue.extend(make_proj_tasks(ch))
            blocks = [(sc, h) for sc in (0, 1) for h in range(H)]
            # staircase split: early blocks take few pairs (only chunks 0-1
            # are projected), late blocks take many (all chunks exist by
            # then), so the final group-B blocks - the tail - are short
            A_CNT = [4, 4, 8, 8, 8, 8, 8, 8, 8, 8, 8, 8, 12, 12, 12, 12]
            for bi, (sc, h) in enumerate(blocks):
                pairs = list(range(A_CNT[bi]))
                if bi == 0:
                    emit_block(h, sc, pairs, True, interleave=c2)
                elif bi == 1:
                    emit_block(h, sc, pairs, True, interleave=c3)
                else:
                    emit_block(h, sc, pairs, True)
                if bi == 1:
                    nc.sync.dma_start(
                        bo_r[:], bo_d[:].unsqueeze(0).to_broadcast((128, 512)))
                    nc.sync.dma_start(wo_s[:], wo_d[:])
                for _ in range(2):
                    if proj_queue:
                        proj_queue.pop(0)()
            while proj_queue:
                proj_queue.pop(0)()
            # group B; spread each sc's output projection into the next sc's
            # blocks (final sc's at the end)
            pending_p3 = []
            for sc in (0, 1):
                for h in range(H):
                    bi = sc * 8 + h
                    bpairs = list(range(A_CNT[bi], NPAIR))
                    emit_block(h, sc, bpairs, False)
                    if pending_p3 and h % 4 == 1:
                        pending_p3.pop(0)()
                pending_p3 = [phase3_task(sc, 0), phase3_task(sc, 1)]
            for t in pending_p3:
                t()
    nc.compile()
    return nc


_NC = None


def _pack_weights(Wq, bq, Wk, bk, Wv, bv, Wo, bo):
    import ml_dtypes
    s = lambda a: np.ascontiguousarray(np.asarray(a, np.float32))
    sb = lambda a: np.ascontiguousarray(
        np.asarray(a, np.float32).astype(ml_dtypes.bfloat16))
    # e-permutation for DoubleRow plane layout: column c = eg*128+p of the
    # stationary maps to head h = p//32 + 4*(eg//2), e = 32*(eg%2) + p%32
    p = np.arange(128)
    eg = np.arange(4)
    hh = p[None, :] // 32 + 4 * (eg[:, None] // 2)     # [4,128]
    ee = 32 * (eg[:, None] % 2) + p[None, :] % 32      # [4,128]

    def pack_qk(W):
        t = np.asarray(W, np.float32)[hh, :, ee]       # [4,128,512(d)]
        t = t.transpose(2, 0, 1)                       # [d, eg, p]
        t = t.reshape(4, 128, 4, 128)                  # [ds, pd, eg, p]
        return sb(t.transpose(1, 0, 2, 3).reshape(128, 4, 512))

    def pack_b(b):
        return s(np.asarray(b, np.float32)[hh, ee].T)  # [128,4]

    wq_p = pack_qk(Wq)
    wk_p = pack_qk(Wk)
    bq_p = pack_b(bq)
    bk_p = pack_b(bk)
    wv_p = sb(np.transpose(Wv, (1, 0, 2)).reshape(D, 512).reshape(4, 128, 512)
              .transpose(1, 0, 2))
    wo_p = s(np.asarray(Wo, np.float32).reshape(8, 64, 512).transpose(1, 0, 2))
    bv_p = s(np.asarray(bv, np.float32).reshape(512))
    bo_p = s(np.asarray(bo, np.float32))
    return dict(wq=wq_p, wk=wk_p, wv=wv_p, wo=wo_p, bq=bq_p, bk=bk_p,
                bv=bv_p, bo=bo_p)


def kernel(x, Wq, bq, Wk, bk, Wv, bv, Wo, bo, **kw):
    global _NC
    x = np.asarray(x, np.float32)
    packed = _pack_weights(Wq, bq, Wk, bk, Wv, bv, Wo, bo)

    if _NC is None:
        _NC = build_program()

    in_maps = []
    for c in range(NCORES):
        b = c // 4
        q0 = (c % 4) * QCHUNK
        xb = np.roll(x[b], -q0, axis=0)  # queries at rows 0:1024
        import ml_dtypes
        m = {"xt": np.ascontiguousarray(xb.T.astype(ml_dtypes.bfloat16))}
        m.update(packed)
        in_maps.append(m)
    res = run_bass_kernel_spmd(_NC, in_maps, core_ids=list(range(NCORES)))
    out = np.empty((B, S, D), np.float32)
    for c in range(NCORES):
        b = c // 4
        q0 = (c % 4) * QCHUNK
        out[b, q0:q0 + QCHUNK] = res.results[c]["out"]
    return out


# revision 56
# speedup vs baseline: 1.0996x; 1.0370x over previous
"""Multi-head attention Trainium2 kernel, 8-core SPMD (v2, fp8 DoubleRow).

Problem: x[2,4096,512], 8 heads of 64; per-head QKV proj, softmax(QK^T/8)V,
concat, output proj.

Sharding: sequence-parallel, no collectives. Core c handles batch b=c//4 and
query rows [1024*(c%4), ...+1024). Each core computes K/V for the full 4096-row
sequence of its batch; x is host-rolled so local query rows sit at 0:1024.

Speed structure:
- x and Q/K/V weights are bf16 (host-converted; halves DMA), projections
  accumulate in f32 PSUM; K/Q converted to fp8-e4m3 with a host-side
  weight-column permutation so the PSUM->SBUF convert is partition-preserving
  and lands directly in the DoubleRow plane layout.
- Scores and PV use fp8-e4m3 DoubleRow matmuls (0.5 cyc/row): score stationary
  kT8[32,2,128] / moving qT8[32,2,512]; PV stationary vA[128,2,66] (a ones
  column provides the softmax denominator; a zero pad column satisfies the
  dual-fp8 Ldweights even-column ISA restriction) / moving pt[128,2,512].
- softmax exp is the wall; it is split across two engines by t-tile pair:
  lane A: ACT native Exp -> fp8 (scale=0.125 fused)
  lane D: DVE tensor_scalar computes e4m3 BITS of exp directly
          (Schraudolph: round(A*s+B) -> int8 -> bitcast fp8e4)
  (Pool/DMA cannot read PSUM, so no third lane is possible.)
- Attention is streamed in two t-groups with a staircase split (A_CNT):
  early blocks run few pairs (only chunks 0-1 are projected yet), late blocks
  run up to 12 so the final group-B tail blocks are short. K/V/Q projection interleaves into
  attention at psc-ring-tile granularity: chunks 2-3 inside the first two
  blocks, chunks 4-7 paced one task per early boundary and two per later
  boundary (early boundaries are already PE-bound); weight DMAs are staged just
  ahead of their first consumer. Per-(h,sc) PV accumulates in one PSUM bank
  per group; group A drains to SBUF (yacc) and is merged + normalized (DVE
  reciprocal + DMA-broadcast + Pool multiply) at group-B block end. PSUM
  budget: 3x2-bank score/projection ring + 2 pav accumulators = 8 banks.
  PV matmuls are emitted 5 pairs behind their scores (6-deep pt rings) so a
  slow exp lane never stalls PE's in-order stream into the next scores.
"""

import numpy as np

import concourse.bass as bass
from concourse import bacc
import concourse.mybir as mybir
import concourse.tile as tile
from concourse.bass_utils import run_bass_kernel_spmd

F32 = mybir.dt.float32
F32R = mybir.dt.float32r
BF16 = mybir.dt.bfloat16
FP8 = mybir.dt.float8e4
I8 = mybir.dt.int8
DR = mybir.MatmulPerfMode.DoubleRow
ADD = mybir.AluOpType.add
MULT = mybir.AluOpType.mult

B, S, D, H, E = 2, 4096, 512, 8, 64
NCORES = 8
QCHUNK = S // 4          # 1024 query rows per core
TCH = 512                # t-rows per projection chunk
NPAIR = S // 256         # 16 t-tile pairs
GROUP_A = list(range(8))        # chunks 0..3
GROUP_B = list(range(8, NPAIR)) # chunks 4..7

A_SCH = 1.442695041      # 8*log2(e)*0.125
B_SCH = 55.632           # calibrated: max rel err 7.2% per weight

# exp lane pattern, cycled over all 256 (h,sc,pair) tasks: A=ACT native exp,
# D=DVE Schraudolph. (Pool cannot read PSUM and DMA cannot read PSUM, so a
# Pool lane would cost DVE the same feeder op as doing the exp on DVE.)
LANES = ("DADAADADAADADAADADAADAADADAADADA"
         "ADADAADADAADAADADAADADAADADAADAA")  # 39 A / 25 D per 64


def build_program():
    nc = bacc.Bacc()
    xt_d = nc.dram_tensor("xt", [D, S], BF16, kind="ExternalInput")
    wq_d = nc.dram_tensor("wq", [128, 4, 512], BF16, kind="ExternalInput")
    wk_d = nc.dram_tensor("wk", [128, 4, 512], BF16, kind="ExternalInput")
    wv_d = nc.dram_tensor("wv", [128, 4, 512], BF16, kind="ExternalInput")
    wo_d = nc.dram_tensor("wo", [64, 8, 512], F32R, kind="ExternalInput")
    bq_d = nc.dram_tensor("bq", [128, 4], F32, kind="ExternalInput")
    bk_d = nc.dram_tensor("bk", [128, 4], F32, kind="ExternalInput")
    bv_d = nc.dram_tensor("bv", [512], F32, kind="ExternalInput")
    bo_d = nc.dram_tensor("bo", [512], F32, kind="ExternalInput")
    out_d = nc.dram_tensor("out", [QCHUNK, D], F32, kind="ExternalOutput")

    with tile.TileContext(nc) as tc:
        with (
            tc.tile_pool(name="const", bufs=1) as cpool,
            tc.tile_pool(name="work", bufs=6) as wpool,
            tc.tile_pool(name="xtp", bufs=3) as xpool,
            tc.tile_pool(name="psc", bufs=3, space="PSUM") as pscpool,
            tc.tile_pool(name="pav", bufs=2, space="PSUM") as pavpool,
            tc.tile_pool(name="dr", bufs=2, space="DRAM") as dpool,
        ):
            wq_s = cpool.tile([128, 4, 512], BF16, tag="wq")
            wk_s = cpool.tile([128, 4, 512], BF16, tag="wk")
            wv_s = cpool.tile([128, 4, 512], BF16, tag="wv")
            wo_s = cpool.tile([64, 8, 512], F32R, tag="wo")
            bq_s = cpool.tile([128, 4], F32, tag="bq")
            bk_s = cpool.tile([128, 4], F32, tag="bk")
            bv_r = cpool.tile([128, 512], F32, tag="bvr")
            bo_r = cpool.tile([128, 512], F32, tag="bor")
            # only K weights up front; the rest stream in between chunk-0
            # tasks so the first projection starts as early as possible
            nc.sync.dma_start(wk_s[:], wk_d[:])
            nc.sync.dma_start(bk_s[:], bk_d[:])

            # fp8 operand tensors
            kT8 = cpool.tile([128, 4, S], FP8, tag="kT8")
            qT8 = cpool.tile([128, 4, QCHUNK], FP8, tag="qT8")
            vA = cpool.tile([128, S // 128, H, E + 2], FP8, tag="vA")
            yacc = cpool.tile([65, 16, 512], F32, tag="yacc")
            yT2 = cpool.tile([64, H, QCHUNK], F32R, tag="yT2")
            nc.vector.memset(vA[:, :, :, E], 1.0)
            nc.vector.memset(vA[:, :, :, E + 1], 0.0)

            def make_proj_tasks(ch):
                """Per-chunk projection split into psc-ring-sized tasks so it
                can interleave with attention blocks at fine grain."""
                state = {}

                def t_x():
                    xT = xpool.tile([128, 4, TCH], BF16, tag="xT")
                    for ds in range(4):
                        nc.sync.dma_start(
                            xT[:, ds, :],
                            xt_d[ds * 128:(ds + 1) * 128,
                                 ch * TCH:(ch + 1) * TCH])
                    state["xT"] = xT

                def t_k(ega):
                    def f():
                        xT = state["xT"]
                        pk = pscpool.tile([128, 2, 512], F32, tag="psc")
                        for i in range(2):
                            eg = ega * 2 + i
                            for ds in range(4):
                                nc.tensor.matmul(
                                    pk[:, i, :],
                                    wk_s[:, ds, eg * 128:(eg + 1) * 128],
                                    xT[:, ds, :], start=(ds == 0),
                                    stop=(ds == 3))
                            nc.scalar.activation(
                                kT8[:, eg, ch * TCH:(ch + 1) * TCH],
                                pk[:, i, :],
                                mybir.ActivationFunctionType.Identity,
                                bias=bk_s[:, eg:eg + 1], scale=1.0)
                    return f

                def t_v(tsa):
                    def f():
                        xT = state["xT"]
                        pv = pscpool.tile([128, 2, 512], F32, tag="psc")
                        for i in range(2):
                            ts = tsa * 2 + i
                            for ds in range(4):
                                nc.tensor.matmul(
                                    pv[:, i, :],
                                    xT[:, ds, ts * 128:(ts + 1) * 128],
                                    wv_s[:, ds, :], start=(ds == 0),
                                    stop=(ds == 3))
                            nc.vector.tensor_tensor(
                                out=vA[:, ch * 4 + ts, :, 0:E],
                                in0=pv[:, i, :].rearrange("p (h e) -> p h e",
                                                          h=H),
                                in1=bv_r[:].rearrange("p (h e) -> p h e", h=H),
                                op=ADD)
                    return f

                def t_q(ega):
                    def f():
                        xT = state["xT"]
                        pq = pscpool.tile([128, 2, 512], F32, tag="psc")
                        for i in range(2):
                            eg = ega * 2 + i
                            for ds in range(4):
                                nc.tensor.matmul(
                                    pq[:, i, :],
                                    wq_s[:, ds, eg * 128:(eg + 1) * 128],
                                    xT[:, ds, :], start=(ds == 0),
                                    stop=(ds == 3))
                            nc.vector.tensor_scalar(
                                qT8[:, eg, ch * TCH:(ch + 1) * TCH],
                                pq[:, i, :], bq_s[:, eg:eg + 1], None, ADD)
                    return f

                def first():
                    t_x()
                    t_k(0)()

                if ch < QCHUNK // TCH:
                    # queries first so attention can start early
                    tasks = [first, t_q(0), t_v(0), t_k(1), t_v(1), t_q(1)]
                else:
                    tasks = [first, t_v(0), t_k(1), t_v(1)]
                return tasks

            def emit_proj_chunk(ch):
                for t in make_proj_tasks(ch):
                    t()

            lane_ctr = [0]

            def emit_block(h, sc, pairs, is_group_a, interleave=None):
                a0 = 32 * (h % 4)
                g0 = 2 * (h // 4)
                n = len(pairs)
                pav = pavpool.tile([128, 512], F32, tag="pav")

                def emit_pv(item):
                    j, tp, ptv = item
                    nc.tensor.matmul(
                        pav[0:66, :], vA[:, 2 * tp:2 * tp + 2, h, :], ptv,
                        start=(j == 0), stop=(j == n - 1), perf_mode=DR)

                pend = []
                for j, tp in enumerate(pairs):
                    lane = LANES[lane_ctr[0] % len(LANES)]
                    lane_ctr[0] += 1
                    psc = pscpool.tile([128, 2, 512], F32, tag="psc")
                    for kt in (0, 1):
                        tt = 2 * tp + kt
                        nc.tensor.matmul(
                            psc[:, kt, :],
                            kT8[a0:a0 + 32, g0:g0 + 2, tt * 128:(tt + 1) * 128],
                            qT8[a0:a0 + 32, g0:g0 + 2, sc * 512:(sc + 1) * 512],
                            start=True, stop=True, perf_mode=DR,
                            tile_position=(a0, 0))
                    if lane == "A":
                        pt = wpool.tile([128, 2, 512], FP8, tag="ptA")
                        nc.scalar.activation(
                            pt[:], psc[:], mybir.ActivationFunctionType.Exp,
                            scale=0.125)
                        ptv = pt[:]
                    elif lane == "D":
                        pti = wpool.tile([128, 2, 512], I8, tag="ptD")
                        nc.vector.tensor_scalar(
                            pti[:], psc[:], A_SCH, B_SCH, MULT, ADD)
                        ptv = pti[:].bitcast(FP8)
                    else:  # lane P: DVE copies PSUM->SBUF bf16, Pool does sch
                        scf = wpool.tile([128, 2, 512], BF16, tag="scf")
                        nc.vector.tensor_copy(scf[:], psc[:])
                        pti = wpool.tile([128, 2, 512], I8, tag="ptP")
                        nc.gpsimd.tensor_scalar(
                            pti[:], scf[:], A_SCH, B_SCH, MULT, ADD)
                        ptv = pti[:].bitcast(FP8)
                    pend.append((j, tp, ptv))
                    if interleave:
                        interleave.pop(0)()
                    if len(pend) == 6:
                        emit_pv(pend.pop(0))
                for item in pend:
                    emit_pv(item)

                slot = sc * 8 + h
                if is_group_a:
                    nc.vector.tensor_copy(yacc[:, slot, :], pav[0:65, :])
                else:
                    tmp = wpool.tile([65, 512], F32, tag="tmp")
                    nc.vector.tensor_tensor(
                        out=tmp[:], in0=yacc[:, slot, :], in1=pav[0:65, :],
                        op=ADD)
                    rec = wpool.tile([1, 512], F32, tag="rec")
                    nc.vector.reciprocal(rec[:], tmp[64:65, :])
                    rec_d = dpool.tile([1, 512], F32, tag="recd")
                    nc.sync.dma_start(rec_d[:], rec[:])
                    rrep = wpool.tile([64, 512], F32, tag="rrep")
                    nc.sync.dma_start(rrep[:], rec_d[:].to_broadcast((64, 512)))
                    nc.gpsimd.tensor_tensor(
                        out=yT2[0:64, h, sc * 512:(sc + 1) * 512],
                        in0=tmp[0:64, :], in1=rrep[:], op=MULT)

            def phase3_task(sc, sta):
                def f():
                    po = pscpool.tile([128, 2, 512], F32, tag="psc")
                    for i in range(2):
                        st = sc * 4 + sta * 2 + i
                        for h in range(H):
                            nc.tensor.matmul(
                                po[:, i, :], yT2[0:64, h, st * 128:(st + 1) * 128],
                                wo_s[0:64, h, :], start=(h == 0), stop=(h == 7))
                        o_s = wpool.tile([128, 512], F32, tag="osb")
                        nc.vector.tensor_tensor(out=o_s[:, :], in0=po[:, i, :],
                                                in1=bo_r[:], op=ADD)
                        nc.sync.dma_start(out_d[st * 128:(st + 1) * 128, :],
                                          o_s[:])
                return f

            # ---- emission ----
            tasks0 = make_proj_tasks(0)
            tasks0[0]()  # x0 DMA + K egs 0,1
            nc.sync.dma_start(wq_s[:], wq_d[:])
            nc.sync.dma_start(bq_s[:], bq_d[:])
            tasks0[1]()  # Q egs 0,1
            nc.sync.dma_start(wv_s[:], wv_d[:])
            nc.sync.dma_start(bv_r[:], bv_d[:].unsqueeze(0).to_broadcast((128, 512)))
            for t in tasks0[2:]:
                t()
            emit_proj_chunk(1)
            c2 = make_proj_tasks(2)
            c3 = make_proj_tasks(3)
            # chunks 4..7 queued as fine-grained tasks, 2 per block boundary
            proj_queue = []
            for ch in range(4, 8):
                proj_queue.extend(make_proj_tasks(ch))
            blocks = [(sc, h) for sc in (0, 1) for h in range(H)]
            # staircase split: early blocks take few pairs (only chunks 0-1
            # are projected), late blocks take many (all chunks exist by
            # then), so the final group-B blocks - the tail - are short
            A_CNT = [4, 4, 8, 8, 8, 8, 8, 8, 8, 8, 8, 8, 12, 12, 12, 12]
            for bi, (sc, h) in enumerate(blocks):
                pairs = list(range(A_CNT[bi]))
                if bi == 0:
                    emit_block(h, sc, pairs, True, interleave=c2)
                elif bi == 1:
                    emit_block(h, sc, pairs, True, interleave=c3)
                else:
                    emit_block(h, sc, pairs, True)
                if bi == 1:
                    nc.sync.dma_start(
                        bo_r[:], bo_d[:].unsqueeze(0).to_broadcast((128, 512)))
                    nc.sync.dma_start(wo_s[:], wo_d[:])
                # back-loaded pacing: early boundaries are already PE-bound
                npop = 1 if bi < 6 else 2
                for _ in range(npop):
                    if proj_queue:
                        proj_queue.pop(0)()
            while proj_queue:
                proj_queue.pop(0)()
            # group B; spread each sc's output projection into the next sc's
            # blocks (final sc's at the end)
            pending_p3 = []
            for sc in (0, 1):
                for h in range(H):
                    bi = sc * 8 + h
                    bpairs = list(range(A_CNT[bi], NPAIR))
                    emit_block(h, sc, bpairs, False)
                    if pending_p3 and h % 4 == 3:
                        pending_p3.pop(0)()
                pending_p3 = [phase3_task(sc, 0), phase3_task(sc, 1)]
            for t in pending_p3:
                t()
    nc.compile()
    return nc


_NC = None


def _pack_weights(Wq, bq, Wk, bk, Wv, bv, Wo, bo):
    import ml_dtypes
    s = lambda a: np.ascontiguousarray(np.asarray(a, np.float32))
    sb = lambda a: np.ascontiguousarray(
        np.asarray(a, np.float32).astype(ml_dtypes.bfloat16))
    # e-permutation for DoubleRow plane layout: column c = eg*128+p of the
    # stationary maps to head h = p//32 + 4*(eg//2), e = 32*(eg%2) + p%32
    p = np.arange(128)
    eg = np.arange(4)
    hh = p[None, :] // 32 + 4 * (eg[:, None] // 2)     # [4,128]
    ee = 32 * (eg[:, None] % 2) + p[None, :] % 32      # [4,128]

    def pack_qk(W):
        t = np.asarray(W, np.float32)[hh, :, ee]       # [4,128,512(d)]
        t = t.transpose(2, 0, 1)                       # [d, eg, p]
        t = t.reshape(4, 128, 4, 128)                  # [ds, pd, eg, p]
        return sb(t.transpose(1, 0, 2, 3).reshape(128, 4, 512))

    def pack_b(b):
        return s(np.asarray(b, np.float32)[hh, ee].T)  # [128,4]

    wq_p = pack_qk(Wq)
    wk_p = pack_qk(Wk)
    bq_p = pack_b(bq)
    bk_p = pack_b(bk)
    wv_p = sb(np.transpose(Wv, (1, 0, 2)).reshape(D, 512).reshape(4, 128, 512)
              .transpose(1, 0, 2))
    wo_p = s(np.asarray(Wo, np.float32).reshape(8, 64, 512).transpose(1, 0, 2))
    bv_p = s(np.asarray(bv, np.float32).reshape(512))
    bo_p = s(np.asarray(bo, np.float32))
    return dict(wq=wq_p, wk=wk_p, wv=wv_p, wo=wo_p, bq=bq_p, bk=bk_p,
                bv=bv_p, bo=bo_p)


def kernel(x, Wq, bq, Wk, bk, Wv, bv, Wo, bo, **kw):
    global _NC
    x = np.asarray(x, np.float32)
    packed = _pack_weights(Wq, bq, Wk, bk, Wv, bv, Wo, bo)

    if _NC is None:
        _NC = build_program()

    in_maps = []
    for c in range(NCORES):
        b = c // 4
        q0 = (c % 4) * QCHUNK
        xb = np.roll(x[b], -q0, axis=0)  # queries at rows 0:1024
        import ml_dtypes
        m = {"xt": np.ascontiguousarray(xb.T.astype(ml_dtypes.bfloat16))}
        m.update(packed)
        in_maps.append(m)
    res = run_bass_kernel_spmd(_NC, in_maps, core_ids=list(range(NCORES)))
    out = np.empty((B, S, D), np.float32)
    for c in range(NCORES):
        b = c // 4
        q0 = (c % 4) * QCHUNK
        out[b, q0:q0 + QCHUNK] = res.results[c]["out"]
    return out


# revision 69
# speedup vs baseline: 1.1043x; 1.0043x over previous
"""Multi-head attention Trainium2 kernel, 8-core SPMD (v2, fp8 DoubleRow).

Problem: x[2,4096,512], 8 heads of 64; per-head QKV proj, softmax(QK^T/8)V,
concat, output proj.

Sharding: sequence-parallel, no collectives. Core c handles batch b=c//4 and
query rows [1024*(c%4), ...+1024). Each core computes K/V for the full 4096-row
sequence of its batch; x is host-rolled so local query rows sit at 0:1024.

Speed structure:
- x and Q/K/V weights are bf16 (host-converted; halves DMA), projections
  accumulate in f32 PSUM; K/Q converted to fp8-e4m3 with a host-side
  weight-column permutation so the PSUM->SBUF convert is partition-preserving
  and lands directly in the DoubleRow plane layout.
- Scores and PV use fp8-e4m3 DoubleRow matmuls (0.5 cyc/row): score stationary
  kT8[32,2,128] / moving qT8[32,2,512]; PV stationary vA[128,2,66] (a ones
  column provides the softmax denominator; a zero pad column satisfies the
  dual-fp8 Ldweights even-column ISA restriction) / moving pt[128,2,512].
- softmax exp is the wall; it is split across two engines by t-tile pair:
  lane A: ACT native Exp -> fp8 (scale=0.125 fused)
  lane D: DVE tensor_scalar computes e4m3 BITS of exp directly
          (Schraudolph: round(A*s+B) -> int8 -> bitcast fp8e4)
  (Pool/DMA cannot read PSUM, so no third lane is possible.)
- Attention is streamed in two t-groups with a staircase split (A_CNT):
  early blocks run few pairs (only chunks 0-1 are projected yet), late blocks
  run up to 12 so the final group-B tail blocks are short. K/V/Q projection interleaves into
  attention at psc-ring-tile granularity: chunks 2-3 inside the first two
  blocks, chunks 4-7 paced one task per early boundary and two per later
  boundary (early boundaries are already PE-bound); weight DMAs are staged just
  ahead of their first consumer. Per-(h,sc) PV accumulates in one PSUM bank
  per group; group A drains to SBUF (yacc) and is merged + normalized (DVE
  reciprocal + DMA-broadcast + Pool multiply) at group-B block end. PSUM
  budget: 3x2-bank score/projection ring + 2 pav accumulators = 8 banks.
  PV matmuls are emitted 5 pairs behind their scores (6-deep pt rings) so a
  slow exp lane never stalls PE's in-order stream into the next scores.
"""

import numpy as np

import concourse.bass as bass
from concourse import bacc
import concourse.mybir as mybir
import concourse.tile as tile
from concourse.bass_utils import run_bass_kernel_spmd

F32 = mybir.dt.float32
F32R = mybir.dt.float32r
BF16 = mybir.dt.bfloat16
FP8 = mybir.dt.float8e4
I8 = mybir.dt.int8
DR = mybir.MatmulPerfMode.DoubleRow
ADD = mybir.AluOpType.add
MULT = mybir.AluOpType.mult

B, S, D, H, E = 2, 4096, 512, 8, 64
NCORES = 8
QCHUNK = S // 4          # 1024 query rows per core
TCH = 512                # t-rows per projection chunk
NPAIR = S // 256         # 16 t-tile pairs
GROUP_A = list(range(8))        # chunks 0..3
GROUP_B = list(range(8, NPAIR)) # chunks 4..7

A_SCH = 1.442695041      # 8*log2(e)*0.125
B_SCH = 55.632           # calibrated: max rel err 7.2% per weight

# exp lane pattern, cycled over all 256 (h,sc,pair) tasks: A=ACT native exp,
# D=DVE Schraudolph. (Pool cannot read PSUM and DMA cannot read PSUM, so a
# Pool lane would cost DVE the same feeder op as doing the exp on DVE.)
LANES = ("DADAADADAADADAADADAADAADADAADADA"
         "ADADAADADAADAADADAADADAADADAADAA")  # 39 A / 25 D per 64


def build_program():
    nc = bacc.Bacc()
    xt_d = nc.dram_tensor("xt", [D, S], BF16, kind="ExternalInput")
    wq_d = nc.dram_tensor("wq", [128, 4, 512], BF16, kind="ExternalInput")
    wk_d = nc.dram_tensor("wk", [128, 4, 512], BF16, kind="ExternalInput")
    wv_d = nc.dram_tensor("wv", [128, 4, 512], BF16, kind="ExternalInput")
    wo_d = nc.dram_tensor("wo", [64, 8, 512], F32R, kind="ExternalInput")
    bq_d = nc.dram_tensor("bq", [128, 4], F32, kind="ExternalInput")
    bk_d = nc.dram_tensor("bk", [128, 4], F32, kind="ExternalInput")
    bv_d = nc.dram_tensor("bv", [512], F32, kind="ExternalInput")
    bo_d = nc.dram_tensor("bo", [512], F32, kind="ExternalInput")
    out_d = nc.dram_tensor("out", [QCHUNK, D], F32, kind="ExternalOutput")

    with tile.TileContext(nc) as tc:
        with (
            tc.tile_pool(name="const", bufs=1) as cpool,
            tc.tile_pool(name="work", bufs=6) as wpool,
            tc.tile_pool(name="xtp", bufs=3) as xpool,
            tc.tile_pool(name="psc", bufs=3, space="PSUM") as pscpool,
            tc.tile_pool(name="pav", bufs=2, space="PSUM") as pavpool,
            tc.tile_pool(name="dr", bufs=2, space="DRAM") as dpool,
        ):
            wq_s = cpool.tile([128, 4, 512], BF16, tag="wq")
            wk_s = cpool.tile([128, 4, 512], BF16, tag="wk")
            wv_s = cpool.tile([128, 4, 512], BF16, tag="wv")
            wo_s = cpool.tile([64, 8, 512], F32R, tag="wo")
            bq_s = cpool.tile([128, 4], F32, tag="bq")
            bk_s = cpool.tile([128, 4], F32, tag="bk")
            bv_r = cpool.tile([128, 512], F32, tag="bvr")
            bo_r = cpool.tile([128, 512], F32, tag="bor")
            # only K weights up front; the rest stream in between chunk-0
            # tasks so the first projection starts as early as possible
            nc.sync.dma_start(wk_s[:], wk_d[:])
            nc.sync.dma_start(bk_s[:], bk_d[:])

            # fp8 operand tensors
            kT8 = cpool.tile([128, 4, S], FP8, tag="kT8")
            qT8 = cpool.tile([128, 4, QCHUNK], FP8, tag="qT8")
            vA = cpool.tile([128, S // 128, H, E + 2], FP8, tag="vA")
            yacc = cpool.tile([65, 16, 512], F32, tag="yacc")
            yT2 = cpool.tile([64, H, QCHUNK], F32R, tag="yT2")
            ones1 = cpool.tile([1, 64], F32, tag="ones1")
            nc.vector.memset(vA[:, :, :, E], 1.0)
            nc.vector.memset(vA[:, :, :, E + 1], 0.0)
            nc.vector.memset(ones1[:], 1.0)

            def make_proj_tasks(ch):
                """Per-chunk projection split into psc-ring-sized tasks so it
                can interleave with attention blocks at fine grain."""
                state = {}

                def t_x():
                    xT = xpool.tile([128, 4, TCH], BF16, tag="xT")
                    for ds in range(4):
                        nc.sync.dma_start(
                            xT[:, ds, :],
                            xt_d[ds * 128:(ds + 1) * 128,
                                 ch * TCH:(ch + 1) * TCH])
                    state["xT"] = xT

                def t_k(ega):
                    def f():
                        xT = state["xT"]
                        pk = pscpool.tile([128, 2, 512], F32, tag="psc")
                        for i in range(2):
                            eg = ega * 2 + i
                            for ds in range(4):
                                nc.tensor.matmul(
                                    pk[:, i, :],
                                    wk_s[:, ds, eg * 128:(eg + 1) * 128],
                                    xT[:, ds, :], start=(ds == 0),
                                    stop=(ds == 3))
                            nc.scalar.activation(
                                kT8[:, eg, ch * TCH:(ch + 1) * TCH],
                                pk[:, i, :],
                                mybir.ActivationFunctionType.Identity,
                                bias=bk_s[:, eg:eg + 1], scale=1.0)
                    return f

                def t_v(tsa):
                    def f():
                        xT = state["xT"]
                        pv = pscpool.tile([128, 2, 512], F32, tag="psc")
                        for i in range(2):
                            ts = tsa * 2 + i
                            for ds in range(4):
                                nc.tensor.matmul(
                                    pv[:, i, :],
                                    xT[:, ds, ts * 128:(ts + 1) * 128],
                                    wv_s[:, ds, :], start=(ds == 0),
                                    stop=(ds == 3))
                            nc.vector.tensor_tensor(
                                out=vA[:, ch * 4 + ts, :, 0:E],
                                in0=pv[:, i, :].rearrange("p (h e) -> p h e",
                                                          h=H),
                                in1=bv_r[:].rearrange("p (h e) -> p h e", h=H),
                                op=ADD)
                    return f

                def t_q(ega):
                    def f():
                        xT = state["xT"]
                        pq = pscpool.tile([128, 2, 512], F32, tag="psc")
                        for i in range(2):
                            eg = ega * 2 + i
                            for ds in range(4):
                                nc.tensor.matmul(
                                    pq[:, i, :],
                                    wq_s[:, ds, eg * 128:(eg + 1) * 128],
                                    xT[:, ds, :], start=(ds == 0),
                                    stop=(ds == 3))
                            nc.vector.tensor_scalar(
                                qT8[:, eg, ch * TCH:(ch + 1) * TCH],
                                pq[:, i, :], bq_s[:, eg:eg + 1], None, ADD)
                    return f

                def first():
                    t_x()
                    t_k(0)()

                if ch < QCHUNK // TCH:
                    # queries first so attention can start early
                    tasks = [first, t_q(0), t_v(0), t_k(1), t_v(1), t_q(1)]
                else:
                    tasks = [first, t_v(0), t_k(1), t_v(1)]
                return tasks

            def emit_proj_chunk(ch):
                for t in make_proj_tasks(ch):
                    t()

            lane_ctr = [0]

            def emit_block(h, sc, pairs, is_group_a, interleave=None):
                a0 = 32 * (h % 4)
                g0 = 2 * (h // 4)
                n = len(pairs)
                pav = pavpool.tile([128, 512], F32, tag="pav")

                def emit_pv(item):
                    j, tp, ptv = item
                    nc.tensor.matmul(
                        pav[0:66, :], vA[:, 2 * tp:2 * tp + 2, h, :], ptv,
                        start=(j == 0), stop=(j == n - 1), perf_mode=DR)

                pend = []
                for j, tp in enumerate(pairs):
                    lane = LANES[lane_ctr[0] % len(LANES)]
                    lane_ctr[0] += 1
                    psc = pscpool.tile([128, 2, 512], F32, tag="psc")
                    for kt in (0, 1):
                        tt = 2 * tp + kt
                        nc.tensor.matmul(
                            psc[:, kt, :],
                            kT8[a0:a0 + 32, g0:g0 + 2, tt * 128:(tt + 1) * 128],
                            qT8[a0:a0 + 32, g0:g0 + 2, sc * 512:(sc + 1) * 512],
                            start=True, stop=True, perf_mode=DR,
                            tile_position=(a0, 0))
                    if lane == "A":
                        pt = wpool.tile([128, 2, 512], FP8, tag="ptA")
                        nc.scalar.activation(
                            pt[:], psc[:], mybir.ActivationFunctionType.Exp,
                            scale=0.125)
                        ptv = pt[:]
                    elif lane == "D":
                        pti = wpool.tile([128, 2, 512], I8, tag="ptD")
                        nc.vector.tensor_scalar(
                            pti[:], psc[:], A_SCH, B_SCH, MULT, ADD)
                        ptv = pti[:].bitcast(FP8)
                    else:  # lane P: DVE copies PSUM->SBUF bf16, Pool does sch
                        scf = wpool.tile([128, 2, 512], BF16, tag="scf")
                        nc.vector.tensor_copy(scf[:], psc[:])
                        pti = wpool.tile([128, 2, 512], I8, tag="ptP")
                        nc.gpsimd.tensor_scalar(
                            pti[:], scf[:], A_SCH, B_SCH, MULT, ADD)
                        ptv = pti[:].bitcast(FP8)
                    pend.append((j, tp, ptv))
                    if interleave:
                        interleave.pop(0)()
                    if len(pend) == 6:
                        emit_pv(pend.pop(0))
                for item in pend:
                    emit_pv(item)

                slot = sc * 8 + h
                if is_group_a:
                    nc.vector.tensor_copy(yacc[:, slot, :], pav[0:65, :])
                else:
                    tmp = wpool.tile([65, 512], F32, tag="tmp")
                    nc.vector.tensor_tensor(
                        out=tmp[:], in0=yacc[:, slot, :], in1=pav[0:65, :],
                        op=ADD)
                    rec = wpool.tile([1, 512], F32, tag="rec")
                    nc.vector.reciprocal(rec[:], tmp[64:65, :])
                    if sc == 1 and h >= 6:
                        # tail: PE broadcast avoids the DMA round-trip latency
                        prr = pavpool.tile([128, 512], F32, tag="pav")
                        nc.tensor.matmul(prr[0:64, :], ones1[:], rec[:],
                                         start=True, stop=True)
                        nc.vector.tensor_tensor(
                            out=yT2[0:64, h, sc * 512:(sc + 1) * 512],
                            in0=tmp[0:64, :], in1=prr[0:64, :], op=MULT)
                    else:
                        rec_d = dpool.tile([1, 512], F32, tag="recd")
                        nc.sync.dma_start(rec_d[:], rec[:])
                        rrep = wpool.tile([64, 512], F32, tag="rrep")
                        nc.sync.dma_start(rrep[:], rec_d[:].to_broadcast((64, 512)))
                        nc.gpsimd.tensor_tensor(
                            out=yT2[0:64, h, sc * 512:(sc + 1) * 512],
                            in0=tmp[0:64, :], in1=rrep[:], op=MULT)

            def phase3_task(sc, sta):
                def f():
                    po = pscpool.tile([128, 2, 512], F32, tag="psc")
                    for i in range(2):
                        st = sc * 4 + sta * 2 + i
                        for h in range(H):
                            nc.tensor.matmul(
                                po[:, i, :], yT2[0:64, h, st * 128:(st + 1) * 128],
                                wo_s[0:64, h, :], start=(h == 0), stop=(h == 7))
                        o_s = wpool.tile([128, 512], F32, tag="osb")
                        nc.vector.tensor_tensor(out=o_s[:, :], in0=po[:, i, :],
                                                in1=bo_r[:], op=ADD)
                        nc.sync.dma_start(out_d[st * 128:(st + 1) * 128, :],
                                          o_s[:])
                return f

            # ---- emission ----
            tasks0 = make_proj_tasks(0)
            tasks0[0]()  # x0 DMA + K egs 0,1
            nc.sync.dma_start(wq_s[:], wq_d[:])
            nc.sync.dma_start(bq_s[:], bq_d[:])
            tasks0[1]()  # Q egs 0,1
            nc.sync.dma_start(wv_s[:], wv_d[:])
            nc.sync.dma_start(bv_r[:], bv_d[:].unsqueeze(0).to_broadcast((128, 512)))
            for t in tasks0[2:]:
                t()
            emit_proj_chunk(1)
            c2 = make_proj_tasks(2)
            c3 = make_proj_tasks(3)
            # chunks 4..7 queued as fine-grained tasks, 2 per block boundary
            proj_queue = []
            for ch in range(4, 8):
                proj_queue.extend(make_proj_tasks(ch))
            blocks = [(sc, h) for sc in (0, 1) for h in range(H)]
            # staircase split: early blocks take few pairs (only chunks 0-1
            # are projected), late blocks take many (all chunks exist by
            # then), so the final group-B blocks - the tail - are short
            A_CNT = [4, 4, 8, 8, 8, 8, 8, 8, 8, 8, 8, 8, 12, 12, 12, 12]
            for bi, (sc, h) in enumerate(blocks):
                pairs = list(range(A_CNT[bi]))
                if bi == 0:
                    emit_block(h, sc, pairs, True, interleave=c2)
                elif bi == 1:
                    emit_block(h, sc, pairs, True, interleave=c3)
                else:
                    emit_block(h, sc, pairs, True)
                if bi == 1:
                    nc.sync.dma_start(
                        bo_r[:], bo_d[:].unsqueeze(0).to_broadcast((128, 512)))
                    nc.sync.dma_start(wo_s[:], wo_d[:])
                # back-loaded pacing: early boundaries are already PE-bound
                npop = 1 if bi < 6 else 2
                for _ in range(npop):
                    if proj_queue:
                        proj_queue.pop(0)()
            while proj_queue:
                proj_queue.pop(0)()
            # group B; spread each sc's output projection into the next sc's
            # blocks (final sc's at the end)
            pending_p3 = []
            for sc in (0, 1):
                for h in range(H):
                    bi = sc * 8 + h
                    bpairs = list(range(A_CNT[bi], NPAIR))
                    emit_block(h, sc, bpairs, False)
                    if pending_p3 and h in (2, 7):
                        pending_p3.pop(0)()
                pending_p3 = [phase3_task(sc, 0), phase3_task(sc, 1)]
            for t in pending_p3:
                t()
    nc.compile()
    return nc


_NC = None


def _pack_weights(Wq, bq, Wk, bk, Wv, bv, Wo, bo):
    import ml_dtypes
    s = lambda a: np.ascontiguousarray(np.asarray(a, np.float32))
    sb = lambda a: np.ascontiguousarray(
        np.asarray(a, np.float32).astype(ml_dtypes.bfloat16))
    # e-permutation for DoubleRow plane layout: column c = eg*128+p of the
    # stationary maps to head h = p//32 + 4*(eg//2), e = 32*(eg%2) + p%32
    p = np.arange(128)
    eg = np.arange(4)
    hh = p[None, :] // 32 + 4 * (eg[:, None] // 2)     # [4,128]
    ee = 32 * (eg[:, None] % 2) + p[None, :] % 32      # [4,128]

    def pack_qk(W):
        t = np.asarray(W, np.float32)[hh, :, ee]       # [4,128,512(d)]
        t = t.transpose(2, 0, 1)                       # [d, eg, p]
        t = t.reshape(4, 128, 4, 128)                  # [ds, pd, eg, p]
        return sb(t.transpose(1, 0, 2, 3).reshape(128, 4, 512))

    def pack_b(b):
        return s(np.asarray(b, np.float32)[hh, ee].T)  # [128,4]

    wq_p = pack_qk(Wq)
    wk_p = pack_qk(Wk)
    bq_p = pack_b(bq)
    bk_p = pack_b(bk)
    wv_p = sb(np.transpose(Wv, (1, 0, 2)).reshape(D, 512).reshape(4, 128, 512)
              .transpose(1, 0, 2))
    wo_p = s(np.asarray(Wo, np.float32).reshape(8, 64, 512).transpose(1, 0, 2))
    bv_p = s(np.asarray(bv, np.float32).reshape(512))
    bo_p = s(np.asarray(bo, np.float32))
    return dict(wq=wq_p, wk=wk_p, wv=wv_p, wo=wo_p, bq=bq_p, bk=bk_p,
                bv=bv_p, bo=bo_p)


def kernel(x, Wq, bq, Wk, bk, Wv, bv, Wo, bo, **kw):
    global _NC
    x = np.asarray(x, np.float32)
    packed = _pack_weights(Wq, bq, Wk, bk, Wv, bv, Wo, bo)

    if _NC is None:
        _NC = build_program()

    in_maps = []
    for c in range(NCORES):
        b = c // 4
        q0 = (c % 4) * QCHUNK
        xb = np.roll(x[b], -q0, axis=0)  # queries at rows 0:1024
        import ml_dtypes
        m = {"xt": np.ascontiguousarray(xb.T.astype(ml_dtypes.bfloat16))}
        m.update(packed)
        in_maps.append(m)
    res = run_bass_kernel_spmd(_NC, in_maps, core_ids=list(range(NCORES)))
    out = np.empty((B, S, D), np.float32)
    for c in range(NCORES):
        b = c // 4
        q0 = (c % 4) * QCHUNK
        out[b, q0:q0 + QCHUNK] = res.results[c]["out"]
    return out
